# revision 1
# baseline (speedup 1.0000x reference)
"""Trainium2 Bass kernel for nn_MultiHeadAttention (dense transformer block).

Reference computation (per batch b of B=2, N=2048 tokens, C=1024, H=16 heads,
D=64 head dim):
    qkv  = x @ W_qkv.T + b_qkv
    q,k,v split into heads; attn = softmax(q @ k.T / sqrt(D)); o = attn @ v
    out  = o @ W_proj.T + b_proj

Sharding over 8 NeuronCores: batch x head-groups.  Core c handles batch
b = c//4 and the 4 heads [4*(c%4), 4*(c%4)+4).  Attention is computed fully
per (batch, head) on one core.  The output projection needs all heads, so
cores AllGather their head-group outputs O^T (f32r) within their 4-core
batch group, then each core computes the full projection for a distinct
512-token slice of its batch.  Host concatenates the 8 slices.

All matmuls run as float32r (full-speed fp32 path on the PE).
"""

import sys

sys.path.insert(0, "/opt/trn_rl_repo")

import numpy as np
import concourse.bass as bass
import concourse.tile as tile
from concourse import mybir, bacc
from concourse.bass_utils import run_bass_kernel_spmd

f32 = mybir.dt.float32
f32r = mybir.dt.float32r

# problem constants (hardcoded per contract)
B = 2
N = 2048
C = 1024
H = 16
D = C // H  # 64
SCALE = D ** -0.5

NCORES = 8
GROUPS = [[0, 1, 2, 3], [4, 5, 6, 7]]
HPC = H // 4  # heads per core = 4
ODC = HPC * D  # per-core o-dim slice = 256
TOKS = N // 4  # output token slice per core = 512


def build_kernel(n=N, c_dim=C, hpc=HPC, ag=True, phases=(1, 2, 4), reps=1):
    """Builds the per-core Bass program. n = sequence length, c_dim = model
    dim, hpc = heads per core (4).  Shapes below follow the real problem when
    defaults are used; smaller n can be used for simulator checks."""
    d = D
    odc = hpc * d                      # 256: per-core o dims
    n_ct = c_dim // 128                # contraction chunks for C
    n_ic = n // 512                    # i (query) chunks of 512
    n_jt = n // 128                    # j (key) tiles of 128
    n_jp = n_jt // 2                   # j tile pairs
    toks = n // 4                      # per-core output token slice
    s_chunks = odc // 128              # 128-wide stationary chunks for q/k/v (2)

    nc = bacc.Bacc("TRN2", target_bir_lowering=False, debug=False,
                   num_devices=NCORES)

    # ---- DRAM I/O ----
    xt = nc.dram_tensor("xt", [c_dim, n], f32r, kind="ExternalInput").ap()
    wq_t = nc.dram_tensor("wq_t", [c_dim, odc], f32r, kind="ExternalInput").ap()
    wk_t = nc.dram_tensor("wk_t", [c_dim, odc], f32r, kind="ExternalInput").ap()
    wv_t = nc.dram_tensor("wv_t", [c_dim, odc], f32r, kind="ExternalInput").ap()
    bqkv = nc.dram_tensor("bqkv", [128, 3 * s_chunks], f32, kind="ExternalInput").ap()
    wp_t = nc.dram_tensor("wp_t", [4 * odc, c_dim], f32r, kind="ExternalInput").ap()
    bp = nc.dram_tensor("bp", [c_dim], f32, kind="ExternalInput").ap()
    y = nc.dram_tensor("y", [toks, c_dim], f32, kind="ExternalOutput").ap()

    with tile.TileContext(nc, pool_alloc_mode="queue") as tc:
        with (
            tc.tile_pool(name="consts", bufs=1) as consts,
            tc.tile_pool(name="qkvsb", bufs=1) as qkvsb,
            tc.tile_pool(name="dram", bufs=1, space="DRAM") as dram,
        ):
            # ---- constants ----
            bqkv_sb = consts.tile([128, 3 * s_chunks], f32)
            nc.sync.dma_start(out=bqkv_sb, in_=bqkv)
            ones32 = consts.tile([128, 1], f32)
            nc.vector.memset(ones32, 1.0)
            ones_r = consts.tile([1, 64], f32r)
            nc.vector.tensor_copy(out=ones_r, in_=ones32[0:1, 0:1].to_broadcast((1, 64)))
            # identity blocks at both partition halves (transpose lhsT base
            # partition must match the identity's)
            ident = consts.tile([128, 64], f32)
            nc.gpsimd.memset(ident, 0.0)
            for half in range(2):
                nc.gpsimd.affine_select(
                    out=ident[half * 64:(half + 1) * 64, :],
                    in_=ident[half * 64:(half + 1) * 64, :],
                    compare_op=mybir.AluOpType.not_equal,
                    fill=1.0, base=0, pattern=[[-1, 64]], channel_multiplier=1,
                )

            # persistent SBUF activations
            qt_sb = qkvsb.tile([128, s_chunks, n], f32r)   # q^T  (head h -> (s=h//2, half=h%2))
            kt_sb = qkvsb.tile([128, s_chunks, n], f32r)   # k^T
            vp_sb = qkvsb.tile([128, n_jt, hpc, 65], f32r)  # v natural + ones col
            ot_sb = qkvsb.tile([128, s_chunks, n], f32r)   # o^T (unnorm->normed)

            # ones column of V'
            nc.vector.tensor_copy(
                out=vp_sb[:, :, :, 64:65],
                in_=ones32[:, 0:1].to_broadcast((128, n_jt, hpc, 1)),
            )

            # ---------- Phase 1: QKV^T projections ----------
            if 1 not in phases:
                for _t in (qt_sb, kt_sb, ot_sb):
                    nc.vector.tensor_copy(out=_t[:, :, 0:1],
                                          in_=ones32[:, 0:1].to_broadcast((128, _t.shape[1], 1)))
                nc.vector.tensor_copy(out=vp_sb[:, 0, :, 0:1],
                                      in_=ones32[:, 0:1].to_broadcast((128, vp_sb.shape[2], 1)))
            if 1 in phases:
              with (
                tc.tile_pool(name="p1w", bufs=1) as p1w,
                tc.tile_pool(name="p1x", bufs=1) as p1x,
                tc.tile_pool(name="p1ps", bufs=3, space="PSUM") as p1ps,
                tc.tile_pool(name="p1tr", bufs=2, space="PSUM") as p1tr,
                tc.tile_pool(name="p1vt", bufs=1) as p1vt,
            ):
                xt_sb = p1x.tile([128, n_ct, n], f32r)
                xt_v = xt.rearrange("(ct p) n -> p ct n", p=128)
                w_sb = {}
                w_vs = {}
                for name, t in (("q", wq_t), ("k", wk_t), ("v", wv_t)):
                    w_sb[name] = p1w.tile([128, n_ct, odc], f32r, name=f"w_{name}")
                    w_vs[name] = t.rearrange("(ct p) m -> p ct m", p=128)
                for ct in range(n_ct):
                    for name in ("q", "k", "v"):
                        nc.sync.dma_start(out=w_sb[name][:, ct, :],
                                          in_=w_vs[name][:, ct, :])
                    nc.sync.dma_start(out=xt_sb[:, ct, :], in_=xt_v[:, ct, :])
                vt_sb = p1vt.tile([128, s_chunks, n], f32)  # v^T staging

                for ti, tname in enumerate(("q", "k", "v")):
                    dst = (qt_sb, kt_sb, vt_sb)[ti]
                    for s in range(s_chunks):
                        bias_col = ti * s_chunks + s
                        for half in range(n // 1024):
                            ps = p1ps.tile([128, 1024], f32, tag="p1ps")
                            for ct in range(n_ct):
                                for n2 in range(2):
                                    nt = half * 2 + n2
                                    nc.tensor.matmul(
                                        ps[:, n2 * 512:(n2 + 1) * 512],
                                        lhsT=w_sb[tname][:, ct, s * 128:(s + 1) * 128],
                                        rhs=xt_sb[:, ct, nt * 512:(nt + 1) * 512],
                                        start=(ct == 0), stop=(ct == n_ct - 1),
                                    )
                            nc.vector.tensor_scalar_add(
                                out=dst[:, s, half * 1024:(half + 1) * 1024],
                                in0=ps,
                                scalar1=bqkv_sb[:, bias_col:bias_col + 1],
                            )

                # transpose V^T -> V natural blocks into vp_sb
                for s in range(s_chunks):  # keep indent
                    for hh in range(2):
                        h_loc = s * 2 + hh
                        for jt in range(n_jt):
                            ptr = p1tr.tile([128, 64], f32, tag="p1tr")
                            nc.tensor.transpose(
                                ptr,
                                in_=vt_sb[hh * 64:(hh + 1) * 64, s,
                                          jt * 128:(jt + 1) * 128],
                                identity=ident[hh * 64:(hh + 1) * 64, :],
                            )
                            nc.vector.tensor_copy(
                                out=vp_sb[:, jt, h_loc, 0:64], in_=ptr
                            )

            # ---------- Phase 2: attention per head ----------
            for _rep in range(reps):
              ag_outs = []
              if 2 in phases:
                with (
                  tc.tile_pool(name="p2s", bufs=2, space="PSUM") as p2s,
                  tc.tile_pool(name="p2u", bufs=2, space="PSUM") as p2u,
                  tc.tile_pool(name="p2b", bufs=2, space="PSUM") as p2b,
                  tc.tile_pool(name="p2e", bufs=6) as p2e,
                  tc.tile_pool(name="p2r", bufs=2) as p2r,
              ):
                  for s in range(s_chunks):
                      for ic in range(n_ic):
                          ps_u = [p2u.tile([65, 512], f32, tag="p2u", name=f"ps_u{_h}") for _h in range(2)]
                          for jp in range(n_jp):
                              e_t = []
                              for hh in range(2):
                                  ps_s = p2s.tile([128, 1024], f32, tag="p2s")
                                  for j2 in range(2):
                                      jt = jp * 2 + j2
                                      nc.tensor.matmul(
                                          ps_s[:, j2 * 512:(j2 + 1) * 512],
                                          lhsT=kt_sb[hh * 64:(hh + 1) * 64, s,
                                                     jt * 128:(jt + 1) * 128],
                                          rhs=qt_sb[hh * 64:(hh + 1) * 64, s,
                                                    ic * 512:(ic + 1) * 512],
                                          start=True, stop=True,
                                      )
                                  e = p2e.tile([128, 1024], f32r, tag="p2e")
                                  nc.scalar.activation(
                                      out=e, in_=ps_s,
                                      func=mybir.ActivationFunctionType.Exp,
                                  )
                                  e_t.append(e)
                              for hh in range(2):
                                  for j2 in range(2):
                                      jt = jp * 2 + j2
                                      nc.tensor.matmul(
                                          ps_u[hh],
                                          lhsT=vp_sb[:, jt, s * 2 + hh, :],
                                          rhs=e_t[hh][:, j2 * 512:(j2 + 1) * 512],
                                          start=(jp == 0 and j2 == 0),
                                          stop=(jp == n_jp - 1 and j2 == 1),
                                      )
                          for hh in range(2):  # normalize
                              r32 = p2r.tile([1, 512], f32, tag="r32")
                              nc.vector.reciprocal(out=r32, in_=ps_u[hh][64:65, :])
                              rr = p2r.tile([1, 512], f32r, tag="rr")
                              nc.vector.tensor_copy(out=rr, in_=r32)
                              ps_b = p2b.tile([64, 512], f32, tag="p2b")
                              nc.tensor.matmul(ps_b, lhsT=ones_r, rhs=rr,
                                               start=True, stop=True)
                              rb_sb = p2r.tile([64, 512], f32, tag="rb_sb")
                              nc.vector.tensor_copy(out=rb_sb, in_=ps_b)
                              nc.vector.tensor_mul(
                                  out=ot_sb[hh * 64:(hh + 1) * 64, s,
                                            ic * 512:(ic + 1) * 512],
                                  in0=ps_u[hh][0:64, :],
                                  in1=rb_sb,
                              )
                      if ag:
                          ag_in_s = dram.tile([128, n], f32r,
                                              name=f"ag_in{s}_{_rep}")
                          nc.sync.dma_start(out=ag_in_s, in_=ot_sb[:, s, :])
                          ag_out_s = dram.tile([512, n], f32r,
                                               name=f"ag_out{s}_{_rep}")
                          nc.gpsimd.collective_compute(
                              "AllGather",
                              mybir.AluOpType.bypass,
                              ins=[ag_in_s[:].opt()],
                              outs=[ag_out_s[:].opt()],
                              replica_groups=GROUPS,
                          )
                          ag_outs.append(ag_out_s)

              # ---------- Phase 3: AllGather O^T ----------
              if 2 not in phases and 4 not in phases:
                  nc.sync.dma_start(out=y.bitcast(f32r), in_=qt_sb[:, :, 0:(toks * c_dim) // (128 * s_chunks)])
                  continue
              if not ag:
                  ag_in = dram.tile([odc, n], f32r)
                  nc.sync.dma_start(
                      out=ag_in.rearrange("(s p) n -> p s n", p=128), in_=ot_sb
                  )
                  ag_out = ag_in

              # ---------- Phase 4: projection on own token slice ----------
              if 4 in phases:
                with (
                  tc.tile_pool(name="p4o", bufs=1) as p4o,
                  tc.tile_pool(name="p4w", bufs=1) as p4w,
                  tc.tile_pool(name="p4ps", bufs=4, space="PSUM") as p4ps,
                  tc.tile_pool(name="p4y", bufs=3) as p4y,
              ):
                  n_od = (4 * odc) // 128 if ag else odc // 128
                  # own token slice: (partition_id % 4) * toks .. +toks
                  ogt = p4o.tile([128, n_od, toks], f32r)
                  pid = nc.partition_id()
                  tok0 = (pid % 4) * toks
                  if ag:
                      ogt_v = ogt.rearrange("p (g s) t -> p g s t", s=s_chunks)
                      for s in range(s_chunks):
                          nc.sync.dma_start(
                              out=ogt_v[:, :, s, :],
                              in_=ag_outs[s].rearrange("(g p) n -> p g n", p=128)[
                                  :, :, bass.ds(tok0, toks)
                              ],
                          )
                  else:
                      nc.sync.dma_start(
                          out=ogt,
                          in_=ag_out.rearrange("(od p) n -> p od n", p=128)[
                              :, :, bass.ds(tok0, toks)
                          ],
                      )
                  wp_sb = p4w.tile([128, n_od, c_dim], f32r)
                  nc.sync.dma_start(
                      out=wp_sb, in_=wp_t.rearrange("(od p) c -> p od c", p=128)[
                          :, 0:n_od, :
                      ]
                  )
                  bp_bc = p4w.tile([128, c_dim], f32)
                  nc.sync.dma_start(
                      out=bp_bc,
                      in_=bass.AP(tensor=bp.tensor, offset=bp.offset,
                                  ap=[[0, 128]] + bp.ap),
                  )
                  for tt in range(toks // 128):
                      ps_y = [p4ps.tile([128, 512], f32, tag="p4ps", name=f"ps_y{_n}") for _n in range(c_dim // 512)]
                      for nc2 in range(c_dim // 512):
                          for od in range(n_od):
                              nc.tensor.matmul(
                                  ps_y[nc2],
                                  lhsT=ogt[:, od, tt * 128:(tt + 1) * 128],
                                  rhs=wp_sb[:, od, nc2 * 512:(nc2 + 1) * 512],
                                  start=(od == 0), stop=(od == n_od - 1),
                              )
                      y_sb = p4y.tile([128, c_dim], f32, tag="y_sb")
                      for nc2 in range(c_dim // 512):
                          nc.vector.tensor_add(
                              out=y_sb[:, nc2 * 512:(nc2 + 1) * 512],
                              in0=ps_y[nc2],
                              in1=bp_bc[:, nc2 * 512:(nc2 + 1) * 512],
                          )
                      nc.sync.dma_start(out=y[tt * 128:(tt + 1) * 128, :], in_=y_sb)

    nc.compile()
    return nc


_CACHE = {}


def _get_nc():
    if "nc" not in _CACHE:
        _CACHE["nc"] = build_kernel()
    return _CACHE["nc"]


def make_in_maps(x, W_qkv, b_qkv, W_proj, b_proj):
    x = np.asarray(x, dtype=np.float32)
    W_qkv = np.asarray(W_qkv, dtype=np.float32)
    b_qkv = np.asarray(b_qkv, dtype=np.float32)
    W_proj = np.asarray(W_proj, dtype=np.float32)
    b_proj = np.asarray(b_proj, dtype=np.float32)

    Wq = W_qkv[0:C] * SCALE
    Wk = W_qkv[C:2 * C]
    Wv = W_qkv[2 * C:3 * C]
    bq = b_qkv[0:C] * SCALE
    bk = b_qkv[C:2 * C]
    bv = b_qkv[2 * C:3 * C]
    wp_t_full = np.ascontiguousarray(W_proj.T)  # [C(od), C]

    # host-side layout prep, deduplicated: x^T is shared by the 4 cores of
    # a batch; weight slices are shared by the 2 cores of a head-group
    xt_by_b = [np.ascontiguousarray(x[b].T) for b in range(B)]  # [C, N]
    per_g = []
    for g in range(4):
        rows = slice(g * ODC, (g + 1) * ODC)
        bcols = np.stack(
            [bq[rows][0:128], bq[rows][128:256],
             bk[rows][0:128], bk[rows][128:256],
             bv[rows][0:128], bv[rows][128:256]], axis=1
        )  # [128, 6]
        per_g.append({
            "wq_t": np.ascontiguousarray(Wq[rows].T),
            "wk_t": np.ascontiguousarray(Wk[rows].T),
            "wv_t": np.ascontiguousarray(Wv[rows].T),
            "bqkv": np.ascontiguousarray(bcols),
        })
    in_maps = []
    for core in range(NCORES):
        b = core // 4
        g = core % 4
        in_maps.append({
            "xt": xt_by_b[b],
            **per_g[g],
            "wp_t": wp_t_full,
            "bp": b_proj,
        })
    return in_maps


def kernel(x, W_qkv, b_qkv, W_proj, b_proj):
    x = np.asarray(x, dtype=np.float32)
    nc = _get_nc()
    in_maps = make_in_maps(x, W_qkv, b_qkv, W_proj, b_proj)
    res = run_bass_kernel_spmd(nc, in_maps, list(range(NCORES)))

    out = np.empty((B, N, C), dtype=np.float32)
    for core in range(NCORES):
        b = core // 4
        g = core % 4
        out[b, g * TOKS:(g + 1) * TOKS, :] = res.results[core]["y"]
    return out



# revision 16
# speedup vs baseline: 1.4367x; 1.4367x over previous
"""Trainium2 Bass kernel for nn_MultiHeadAttention (dense transformer block).

Reference computation (B=2 batches, N=2048 tokens, C=1024, H=16 heads, D=64):
    qkv  = x @ W_qkv.T + b_qkv
    q,k,v split into heads; attn = softmax(q @ k.T / sqrt(D)); o = attn @ v
    out  = o @ W_proj.T + b_proj

Sharding over 8 NeuronCores: head-parallel attention, token-parallel
projection.  Core c owns heads {2c, 2c+1} and computes QKV + attention for
both batches for those heads.  The per-head outputs o^T are exchanged with a
single 8-way AllToAll per head (each core sends, for every peer r, its head's
o^T slice for peer r's (batch, token-slice)); afterwards each core holds
o^T of ALL 16 heads for its own 512-token slice (batch c//4, tokens
(c%4)*512..) and computes the full output projection there.

All matmuls run in bf16 (fp32 PSUM accumulation).  attn@V uses the exp tile
as the stationary operand and V (with an appended ones column) as the moving
operand, so each matmul streams only 65 rows at full 128x128 PE utilization
and the softmax denominator lands on the same PSUM partition as the outputs
(normalization = per-partition scalar multiply on the vector engine).
"""

import sys

sys.path.insert(0, "/opt/trn_rl_repo")

import numpy as np
import ml_dtypes
import concourse.bass as bass
import concourse.tile as tile
from concourse import mybir, bacc
from concourse.bass_utils import run_bass_kernel_spmd

f32 = mybir.dt.float32
bf16 = mybir.dt.bfloat16
f8 = mybir.dt.float8e4
np_bf16 = ml_dtypes.bfloat16

# problem constants (hardcoded per contract)
B = 2
N = 2048
C = 1024
H = 16
D = C // H  # 64
SCALE = D ** -0.5

NCORES = 8
GROUPS8 = [[0, 1, 2, 3, 4, 5, 6, 7]]
HPC = H // NCORES          # heads per core = 2
TOKS = N // 4              # per-core output token slice = 512
N_CT = C // 128            # contraction chunks over C = 8
N_JT = N // 128            # key tiles = 16
N_IC = N // 512            # query blocks = 4
N_OD = C // 128            # o-dim contraction chunks in proj = 8


def build_kernel():
    nc = bacc.Bacc("TRN2", target_bir_lowering=False, debug=False,
                   num_devices=NCORES)

    # ---- DRAM I/O (all bf16 except biases / final output) ----
    xt = nc.dram_tensor("xt", [B, C, N], bf16, kind="ExternalInput").ap()
    # fused [k|q|v] weight slab: [ct, 128 part, 3*128] -> single DMA
    wkqv = nc.dram_tensor("wkqv", [N_CT, 128, 3 * 128], bf16,
                          kind="ExternalInput").ap()
    bqk = nc.dram_tensor("bqk", [128, 2], f32, kind="ExternalInput").ap()
    bv = nc.dram_tensor("bv", [128], bf16, kind="ExternalInput").ap()
    wp_t = nc.dram_tensor("wp_t", [C, C], bf16, kind="ExternalInput").ap()
    bp = nc.dram_tensor("bp", [C], f32, kind="ExternalInput").ap()
    y = nc.dram_tensor("y", [TOKS, C], f32, kind="ExternalOutput").ap()

    with tile.TileContext(nc, pool_alloc_mode="queue") as tc:
        with (
            tc.tile_pool(name="consts", bufs=1) as consts,
            tc.tile_pool(name="persist", bufs=1) as persist,
            tc.tile_pool(name="p1x", bufs=1) as p1x,
            tc.tile_pool(name="p1w", bufs=1) as p1w,
            tc.tile_pool(name="epool", bufs=18) as epool,
            tc.tile_pool(name="opool", bufs=6) as opool,
            tc.tile_pool(name="rpool", bufs=4) as rpool,
            tc.tile_pool(name="ypool", bufs=3) as ypool,
            tc.tile_pool(name="ps_s", bufs=2, space="PSUM") as ps_s_pool,
            tc.tile_pool(name="ps_o", bufs=1, space="PSUM") as ps_o_pool,
            tc.tile_pool(name="misc", bufs=3, space="PSUM") as misc,
            tc.tile_pool(name="dram", bufs=1, space="DRAM") as dram,
        ):
            # ---------------- constants ----------------
            bqk_sb = consts.tile([128, 2], f32)
            nc.sync.dma_start(out=bqk_sb, in_=bqk)
            # V bias broadcast: [128 part, jt-dup 2, h 2, d 64]
            bv_bc = consts.tile([128, 2, 2, 64], bf16)
            nc.sync.dma_start(
                out=bv_bc,
                in_=bass.AP(tensor=bv.tensor, offset=bv.offset,
                            ap=[[0, 128], [0, 2], [64, 2], [1, 64]]),
            )
            bp_bc = consts.tile([128, C], f32)
            # exp shift (keeps fp8 exp in range; cancels in softmax ratio)
            eshift = consts.tile([128, 1], f32)
            nc.vector.memset(eshift, -4.0)
            # 128x128 bf16 identity (moving operand of PE transposes)
            ident = consts.tile([128, 128], bf16)
            nc.gpsimd.memset(ident, 0.0)
            nc.gpsimd.affine_select(
                out=ident, in_=ident,
                compare_op=mybir.AluOpType.not_equal,
                fill=1.0, base=0, pattern=[[-1, 128]], channel_multiplier=1,
            )

            # -------------- persistent activations --------------
            # partition dim = 2 local heads x 64 dims (bf16: fp8 attention
            # was tested and fails the 2e-2 tolerance - the near-diagonal
            # logits reach 9.0, so rows are peaked and quantization noise
            # on q/k/e/v transfers directly into the output)
            qt_sb = persist.tile([128, B, N], bf16)   # q^T
            kt_sb = persist.tile([128, B, N], bf16)   # k^T
            # V natural + ones column: [tok-part, b, jt, h, 65]
            vp_sb = persist.tile([128, B, N_JT, HPC, 65], bf16)
            nc.vector.memset(vp_sb[:, :, :, :, 64:65], 1.0)
            # o^T (normalized): [64 dims, head, b, t] - 64-partition tile so
            # all engine copies into it stay partition-base aligned
            ot_sb = persist.tile([64, HPC, B, N], bf16)

            # weight / x staging
            xt_sb = p1x.tile([128, N_CT, N], bf16)
            wkqv_sb = p1w.tile([128, N_CT, 3 * 128], bf16)
            wk_sb = wkqv_sb[:, :, 0:128]
            wq_sb = wkqv_sb[:, :, 128:256]
            wv_sb = wkqv_sb[:, :, 256:384]
            wp_sb = p1w.tile([128, N_OD, C], bf16)
            ogt_sb = persist.tile([128, N_OD, TOKS], bf16)

            xt_views = [
                xt[b].rearrange("(ct p) n -> p ct n", p=128) for b in range(B)
            ]

            def emit_xt_dma(b):
                for ct in range(N_CT):
                    nc.sync.dma_start(out=xt_sb[:, ct, :],
                                      in_=xt_views[b][:, ct, :])

            # W_kqv (one instruction) + x^T(b0); W_p deferred until the
            # attention phase is underway (it is first read ~150us in).
            nc.sync.dma_start(out=wkqv_sb,
                              in_=wkqv.rearrange("ct p m -> p ct m"))
            emit_xt_dma(0)

            def emit_qk(b, tensor, segs):
                """q^T / k^T projection for 512-token segments `segs`."""
                w_sb = wq_sb if tensor == "q" else wk_sb
                bcol = 0 if tensor == "q" else 1
                for seg in segs:
                    ps = misc.tile([128, 512], f32, tag="mpsum")
                    for ct in range(N_CT):
                        nc.tensor.matmul(
                            ps,
                            lhsT=w_sb[:, ct, :],
                            rhs=xt_sb[:, ct, seg * 512:(seg + 1) * 512],
                            start=(ct == 0), stop=(ct == N_CT - 1),
                        )
                    sl = slice(seg * 512, (seg + 1) * 512)
                    dst = qt_sb[:, b, sl] if tensor == "q" else kt_sb[:, b, sl]
                    nc.vector.tensor_scalar_add(
                        out=dst,
                        in0=ps,
                        scalar1=bqk_sb[:, bcol:bcol + 1],
                    )

            def emit_v(b, pairs):
                """V natural projection for pairs of 128-token tiles."""
                for p in pairs:
                    ps = misc.tile([128, 2, HPC, 64], f32, tag="mpsum")
                    for g in range(2):
                        tt = 2 * p + g
                        for ct in range(N_CT):
                            nc.tensor.matmul(
                                ps[:, g, :, :],
                                lhsT=xt_sb[:, ct, tt * 128:(tt + 1) * 128],
                                rhs=wv_sb[:, ct, :],
                                start=(ct == 0), stop=(ct == N_CT - 1),
                            )
                    nc.vector.tensor_add(
                        out=vp_sb[:, b, 2 * p:2 * p + 2, :, 0:64],
                        in0=ps,
                        in1=bv_bc,
                    )

            def emit_scores_exp(s, b, ic):
                """scores + exp for head s, batch b, 512-query block ic.
                Returns the 8 exp tiles ([128 keys, 2 jt x 512 q] each)."""
                e_tiles = []
                for jp in range(8):
                    ps = ps_s_pool.tile([128, 1024], f32, tag="ps_s")
                    for j2 in range(2):
                        jt = jp * 2 + j2
                        nc.tensor.matmul(
                            ps[:, j2 * 512:(j2 + 1) * 512],
                            lhsT=kt_sb[64 * s:64 * (s + 1), b,
                                       jt * 128:(jt + 1) * 128],
                            rhs=qt_sb[64 * s:64 * (s + 1), b,
                                      ic * 512:(ic + 1) * 512],
                            start=True, stop=True,
                        )
                    # constant shift (cancels in the softmax ratio) keeps the
                    # largest exp values small; softmax scale applied here in
                    # f32 rather than folded into quantized weights
                    e = epool.tile([128, 1024], bf16, tag="e")
                    nc.scalar.activation(
                        out=e, in_=ps,
                        func=mybir.ActivationFunctionType.Exp,
                        scale=SCALE, bias=eshift[:, 0:1],
                    )
                    e_tiles.append(e)
                return e_tiles

            def emit_attnv(s, b, ic, e_tiles):
                """attn@V + normalize + transpose into ot_sb."""
                ps_o = ps_o_pool.tile([128, 4, 65], f32, tag="ps_o")
                for qt in range(4):
                    for jt in range(N_JT):
                        e = e_tiles[jt // 2]
                        qoff = (jt % 2) * 512 + qt * 128
                        nc.tensor.matmul(
                            ps_o[:, qt, :],
                            lhsT=e[:, qoff:qoff + 128],
                            rhs=vp_sb[:, b, jt, s, :],
                            start=(jt == 0), stop=(jt == N_JT - 1),
                        )
                r = rpool.tile([128, 4, 1], f32, tag="r")
                nc.vector.reciprocal(out=r, in_=ps_o[:, :, 64:65])
                for qt in range(4):
                    o_t = opool.tile([128, 64], bf16, tag="o")
                    nc.vector.tensor_scalar_mul(
                        out=o_t, in0=ps_o[:, qt, 0:64], scalar1=r[:, qt, :],
                    )
                    tp = misc.tile([64, 128], bf16, tag="mpsum")
                    nc.tensor.transpose(tp, in_=o_t, identity=ident)
                    nc.vector.tensor_copy(
                        out=ot_sb[:, s, b,
                                  ic * 512 + qt * 128:ic * 512 + (qt + 1) * 128],
                        in_=tp,
                    )

            # partial projection accumulator (head-1 od chunks, + bias)
            y_acc = persist.tile([128, 4, C], f32)

            def emit_at(s):
                """8-way AllToAll of head s's o^T + landing DMA into ogt."""
                at_in_s = dram.tile([512, 512], bf16, name=f"at_in{s}")
                nc.sync.dma_start(
                    out=at_in_s.rearrange("(b g p) t -> p b g t", p=64, b=B),
                    in_=ot_sb[:, s, :, :].rearrange(
                        "p b (g t) -> p b g t", g=4),
                )
                at_out_s = dram.tile([512, 512], bf16, name=f"at_out{s}")
                nc.gpsimd.collective_compute(
                    "AllToAll",
                    mybir.AluOpType.bypass,
                    ins=[at_in_s[:].opt()],
                    outs=[at_out_s[:].opt()],
                    replica_groups=GROUPS8,
                )
                nc.sync.dma_start(
                    out=ogt_sb[:, s * 4:(s + 1) * 4, :],
                    in_=at_out_s.rearrange("(o p) t -> p o t", p=128),
                )

            def emit_partial_proj(tt):
                """proj over head-1 od chunks (4..7) into y_acc, + bias."""
                for nc2 in range(2):
                    ps = misc.tile([128, 512], f32, tag="mpsum",
                                   name=f"ps_pp{tt}_{nc2}")
                    for i, od in enumerate(range(4, 8)):
                        nc.tensor.matmul(
                            ps,
                            lhsT=ogt_sb[:, od, tt * 128:(tt + 1) * 128],
                            rhs=wp_sb[:, od, nc2 * 512:(nc2 + 1) * 512],
                            start=(i == 0), stop=(i == 3),
                        )
                    nc.vector.tensor_add(
                        out=y_acc[:, tt, nc2 * 512:(nc2 + 1) * 512],
                        in0=ps,
                        in1=bp_bc[:, nc2 * 512:(nc2 + 1) * 512],
                    )

            # ---------------- emission schedule ----------------
            # Unit order: (h0,b0), (h1,b0), (h1,b1), (h0,b1).
            # AT(h1) fires after unit 2 and hides under unit 3; partial
            # projection of the h1 od-chunks runs inside unit 3's blocks;
            # only AT(h0) + the h0 half of proj remain at the end.
            units = [(0, 0), (1, 0), (1, 1), (0, 1)]
            # per-(unit,ic) PE work interleaved into the blocks.  "pre"
            # slots produce data the NEXT block's scores need (q segments)
            # and run before the lookahead scores; "post" slots (v, weight
            # DMAs, ...) run after them, before attnV of the current block.
            slots_pre = {
                (0, 0): [lambda: emit_qk(0, "q", [1])],
                (0, 1): [lambda: emit_qk(0, "q", [2])],
                (0, 2): [lambda: emit_qk(0, "q", [3])],
            }
            slots_post = {
                # v(b0) must be fully emitted before unit 0's first attnV
                (0, 0): [lambda: emit_v(0, range(8))],
                (0, 3): [lambda: emit_xt_dma(1)],
                (1, 0): [lambda: emit_qk(1, "k", range(2))],
                (1, 1): [lambda: emit_qk(1, "k", range(2, 4)),
                         lambda: emit_qk(1, "q", [0]),
                         lambda: nc.sync.dma_start(
                             out=wp_sb,
                             in_=wp_t.rearrange("(od p) c -> p od c", p=128)),
                         lambda: nc.sync.dma_start(
                             out=bp_bc,
                             in_=bass.AP(tensor=bp.tensor, offset=bp.offset,
                                         ap=[[0, 128]] + bp.ap))],
                (1, 2): [lambda: emit_qk(1, "q", range(1, 3))],
                (1, 3): [lambda: emit_qk(1, "q", [3]),
                         lambda: emit_v(1, range(3))],
                (2, 0): [lambda: emit_v(1, range(3, 8))],
            }

            emit_qk(0, "k", range(4))
            emit_qk(0, "q", [0])

            # flat block list, software-pipelined one block ahead: scores+exp
            # for block j+1 are emitted before attnV of block j, so the
            # activation engine never waits out attnV/transposes at block and
            # unit boundaries.
            blocks = [(ui, s, b, ic)
                      for ui, (s, b) in enumerate(units)
                      for ic in range(N_IC)]
            e_cur = emit_scores_exp(*blocks[0][1:])
            for j, (ui, s, b, ic) in enumerate(blocks):
                for work in slots_pre.get((ui, ic), []):
                    work()
                if j + 1 < len(blocks):
                    e_next = emit_scores_exp(*blocks[j + 1][1:])
                for work in slots_post.get((ui, ic), []):
                    work()
                emit_attnv(s, b, ic, e_cur)
                e_cur = e_next
                if ic == N_IC - 1:
                    if ui == 2:
                        emit_at(1)
                    elif ui == 3:
                        emit_at(0)

            # partial projection over the h1 od chunks (landed with AT(1))
            # overlaps AT(0)'s transfer
            for tt in range(TOKS // 128):
                emit_partial_proj(tt)

            # PE warmer: keep the tensor engine clocked up through the AT(0)
            # wait so the final projection issues at full p-state instead of
            # restarting from the low-clock ramp after ~25us of idle.
            warm = misc.tile([128, 512], f32, tag="mpsum", name="warm")
            for _ in range(96):
                nc.tensor.matmul(warm, lhsT=ogt_sb[:, 4, 0:128],
                                 rhs=wp_sb[:, 4, 0:512],
                                 start=True, stop=True)

            # ---------------- final projection (head-0 od chunks) ----------
            # runs after AT(0) lands; uses the attention-phase ps_s pool
            # (free by now) so each token tile needs one psum tile and one
            # wide DVE add
            for tt in range(TOKS // 128):
                ps_y = ps_s_pool.tile([128, 1024], f32, tag="ps_s",
                                      name=f"ps_y{tt}")
                for nc2 in range(2):
                    for od in range(4):
                        nc.tensor.matmul(
                            ps_y[:, nc2 * 512:(nc2 + 1) * 512],
                            lhsT=ogt_sb[:, od, tt * 128:(tt + 1) * 128],
                            rhs=wp_sb[:, od, nc2 * 512:(nc2 + 1) * 512],
                            start=(od == 0), stop=(od == 3),
                        )
                y_sb = ypool.tile([128, C], f32, tag="y_sb")
                nc.vector.tensor_add(
                    out=y_sb, in0=ps_y, in1=y_acc[:, tt, :],
                )
                nc.sync.dma_start(out=y[tt * 128:(tt + 1) * 128, :], in_=y_sb)

    nc.compile()
    return nc


_CACHE = {}


def _get_nc():
    if "nc" not in _CACHE:
        _CACHE["nc"] = build_kernel()
    return _CACHE["nc"]


def make_in_maps(x, W_qkv, b_qkv, W_proj, b_proj):
    x = np.asarray(x, dtype=np.float32)
    W_qkv = np.asarray(W_qkv, dtype=np.float32)
    b_qkv = np.asarray(b_qkv, dtype=np.float32)
    W_proj = np.asarray(W_proj, dtype=np.float32)
    b_proj = np.asarray(b_proj, dtype=np.float32)

    Wq = W_qkv[0:C]
    Wk = W_qkv[C:2 * C]
    Wv = W_qkv[2 * C:3 * C]
    bq = b_qkv[0:C]
    bk = b_qkv[C:2 * C]
    bv_full = b_qkv[2 * C:3 * C]

    # x^T for both batches, shared by all cores
    xtb = np.ascontiguousarray(
        x.transpose(0, 2, 1)).astype(np_bf16)  # [B, C, N]

    # W_proj.T with rows permuted to the AllToAll arrival order:
    # od chunk (s, o) holds heads {4o+s, 4o+2+s}
    perm = []
    for s in range(HPC):
        for o in range(4):
            for h in (4 * o + s, 4 * o + 2 + s):
                perm.extend(range(64 * h, 64 * (h + 1)))
    wp_t_full = np.ascontiguousarray(W_proj.T[perm, :]).astype(np_bf16)

    in_maps = []
    for core in range(NCORES):
        rows = slice(128 * core, 128 * (core + 1))  # dims of heads 2c, 2c+1
        # fused [k|q|v] weight slab in [ct, 128, 384] layout
        slab = np.concatenate(
            [Wk[rows].T, Wq[rows].T, Wv[rows].T], axis=1)  # [C, 384]
        slab = np.ascontiguousarray(
            slab.reshape(N_CT, 128, 3 * 128)).astype(np_bf16)
        in_maps.append({
            "xt": xtb,
            "wkqv": slab,
            "bqk": np.ascontiguousarray(
                np.stack([bq[rows], bk[rows]], axis=1)),
            "bv": bv_full[rows].astype(np_bf16),
            "wp_t": wp_t_full,
            "bp": b_proj,
        })
    return in_maps


def kernel(x, W_qkv, b_qkv, W_proj, b_proj):
    nc = _get_nc()
    in_maps = make_in_maps(x, W_qkv, b_qkv, W_proj, b_proj)
    res = run_bass_kernel_spmd(nc, in_maps, list(range(NCORES)))

    out = np.empty((B, N, C), dtype=np.float32)
    for core in range(NCORES):
        b = core // 4
        g = core % 4
        out[b, g * TOKS:(g + 1) * TOKS, :] = res.results[core]["y"]
    return out


# revision 20
# speedup vs baseline: 1.4691x; 1.0225x over previous
"""Trainium2 Bass kernel for nn_MultiHeadAttention (dense transformer block).

Reference computation (B=2 batches, N=2048 tokens, C=1024, H=16 heads, D=64):
    qkv  = x @ W_qkv.T + b_qkv
    q,k,v split into heads; attn = softmax(q @ k.T / sqrt(D)); o = attn @ v
    out  = o @ W_proj.T + b_proj

Sharding over 8 NeuronCores: head-parallel attention, token-parallel
projection.  Core c owns heads {2c, 2c+1} and computes QKV + attention for
both batches for those heads.  The per-head outputs o^T are exchanged with a
single 8-way AllToAll per head (each core sends, for every peer r, its head's
o^T slice for peer r's (batch, token-slice)); afterwards each core holds
o^T of ALL 16 heads for its own 512-token slice (batch c//4, tokens
(c%4)*512..) and computes the full output projection there.

All matmuls run in bf16 (fp32 PSUM accumulation).  attn@V uses the exp tile
as the stationary operand and V (with an appended ones column) as the moving
operand, so each matmul streams only 65 rows at full 128x128 PE utilization
and the softmax denominator lands on the same PSUM partition as the outputs
(normalization = per-partition scalar multiply on the vector engine).
"""

import sys

sys.path.insert(0, "/opt/trn_rl_repo")

import numpy as np
import ml_dtypes
import concourse.bass as bass
import concourse.tile as tile
from concourse import mybir, bacc
from concourse.bass_utils import run_bass_kernel_spmd

f32 = mybir.dt.float32
bf16 = mybir.dt.bfloat16
f8 = mybir.dt.float8e4
np_bf16 = ml_dtypes.bfloat16

# problem constants (hardcoded per contract)
B = 2
N = 2048
C = 1024
H = 16
D = C // H  # 64
SCALE = D ** -0.5

NCORES = 8
GROUPS8 = [[0, 1, 2, 3, 4, 5, 6, 7]]
HPC = H // NCORES          # heads per core = 2
TOKS = N // 4              # per-core output token slice = 512
N_CT = C // 128            # contraction chunks over C = 8
N_JT = N // 128            # key tiles = 16
N_IC = N // 512            # query blocks = 4
N_OD = C // 128            # o-dim contraction chunks in proj = 8


def build_kernel():
    nc = bacc.Bacc("TRN2", target_bir_lowering=False, debug=False,
                   num_devices=NCORES)

    # ---- DRAM I/O (all bf16 except biases / final output) ----
    xt = nc.dram_tensor("xt", [B, C, N], bf16, kind="ExternalInput").ap()
    # fused [k|q|v] weight slab: [ct, 128 part, 3*128] -> single DMA
    wkqv = nc.dram_tensor("wkqv", [N_CT, 128, 3 * 128], bf16,
                          kind="ExternalInput").ap()
    bqk = nc.dram_tensor("bqk", [128, 2], f32, kind="ExternalInput").ap()
    bv = nc.dram_tensor("bv", [128], bf16, kind="ExternalInput").ap()
    wp_t = nc.dram_tensor("wp_t", [C, C], bf16, kind="ExternalInput").ap()
    bp = nc.dram_tensor("bp", [C], f32, kind="ExternalInput").ap()
    y = nc.dram_tensor("y", [TOKS, C], f32, kind="ExternalOutput").ap()

    with tile.TileContext(nc, pool_alloc_mode="queue") as tc:
        with (
            tc.tile_pool(name="consts", bufs=1) as consts,
            tc.tile_pool(name="persist", bufs=1) as persist,
            tc.tile_pool(name="p1x", bufs=1) as p1x,
            tc.tile_pool(name="p1w", bufs=1) as p1w,
            tc.tile_pool(name="epool", bufs=18) as epool,
            tc.tile_pool(name="opool", bufs=6) as opool,
            tc.tile_pool(name="rpool", bufs=4) as rpool,
            tc.tile_pool(name="ypool", bufs=3) as ypool,
            tc.tile_pool(name="ps_s", bufs=2, space="PSUM") as ps_s_pool,
            tc.tile_pool(name="ps_o", bufs=1, space="PSUM") as ps_o_pool,
            tc.tile_pool(name="misc", bufs=3, space="PSUM") as misc,
            tc.tile_pool(name="dram", bufs=1, space="DRAM") as dram,
        ):
            # ---------------- constants ----------------
            bqk_sb = consts.tile([128, 2], f32)
            nc.sync.dma_start(out=bqk_sb, in_=bqk)
            # V bias broadcast: [128 part, jt-dup 2, h 2, d 64]
            bv_bc = consts.tile([128, 2, 2, 64], bf16)
            nc.sync.dma_start(
                out=bv_bc,
                in_=bass.AP(tensor=bv.tensor, offset=bv.offset,
                            ap=[[0, 128], [0, 2], [64, 2], [1, 64]]),
            )
            bp_bc = consts.tile([128, C], f32)
            # exp shift (keeps fp8 exp in range; cancels in softmax ratio)
            eshift = consts.tile([128, 1], f32)
            nc.vector.memset(eshift, -4.0)
            # 128x128 bf16 identity (moving operand of PE transposes)
            ident = consts.tile([128, 128], bf16)
            nc.gpsimd.memset(ident, 0.0)
            nc.gpsimd.affine_select(
                out=ident, in_=ident,
                compare_op=mybir.AluOpType.not_equal,
                fill=1.0, base=0, pattern=[[-1, 128]], channel_multiplier=1,
            )

            # -------------- persistent activations --------------
            # partition dim = 2 local heads x 64 dims (bf16: fp8 attention
            # was tested and fails the 2e-2 tolerance - the near-diagonal
            # logits reach 9.0, so rows are peaked and quantization noise
            # on q/k/e/v transfers directly into the output)
            qt_sb = persist.tile([128, B, N], bf16)   # q^T
            kt_sb = persist.tile([128, B, N], bf16)   # k^T
            # V natural + ones column: [tok-part, b, jt, h, 65]
            vp_sb = persist.tile([128, B, N_JT, HPC, 65], bf16)
            nc.vector.memset(vp_sb[:, :, :, :, 64:65], 1.0)
            # o^T (normalized): [64 dims, head, b, t] - 64-partition tile so
            # all engine copies into it stay partition-base aligned
            ot_sb = persist.tile([64, HPC, B, N], bf16)

            # weight / x staging
            xt_sb = p1x.tile([128, N_CT, N], bf16)
            wkqv_sb = p1w.tile([128, N_CT, 3 * 128], bf16)
            wk_sb = wkqv_sb[:, :, 0:128]
            wq_sb = wkqv_sb[:, :, 128:256]
            wv_sb = wkqv_sb[:, :, 256:384]
            wp_sb = p1w.tile([128, N_OD, C], bf16)
            ogt_sb = persist.tile([128, N_OD, TOKS], bf16)

            xt_views = [
                xt[b].rearrange("(ct p) n -> p ct n", p=128) for b in range(B)
            ]

            def emit_xt_dma(b):
                for ct in range(N_CT):
                    nc.sync.dma_start(out=xt_sb[:, ct, :],
                                      in_=xt_views[b][:, ct, :])

            # W_kqv (one instruction) + x^T(b0); W_p deferred until the
            # attention phase is underway (it is first read ~150us in).
            nc.sync.dma_start(out=wkqv_sb,
                              in_=wkqv.rearrange("ct p m -> p ct m"))
            emit_xt_dma(0)

            def emit_qk(b, tensor, segs):
                """q^T / k^T projection for 512-token segments `segs`."""
                w_sb = wq_sb if tensor == "q" else wk_sb
                bcol = 0 if tensor == "q" else 1
                for seg in segs:
                    ps = misc.tile([128, 512], f32, tag="mpsum")
                    for ct in range(N_CT):
                        nc.tensor.matmul(
                            ps,
                            lhsT=w_sb[:, ct, :],
                            rhs=xt_sb[:, ct, seg * 512:(seg + 1) * 512],
                            start=(ct == 0), stop=(ct == N_CT - 1),
                        )
                    sl = slice(seg * 512, (seg + 1) * 512)
                    dst = qt_sb[:, b, sl] if tensor == "q" else kt_sb[:, b, sl]
                    nc.vector.tensor_scalar_add(
                        out=dst,
                        in0=ps,
                        scalar1=bqk_sb[:, bcol:bcol + 1],
                    )

            def emit_v(b, pairs):
                """V natural projection for pairs of 128-token tiles."""
                for p in pairs:
                    ps = misc.tile([128, 2, HPC, 64], f32, tag="mpsum")
                    for g in range(2):
                        tt = 2 * p + g
                        for ct in range(N_CT):
                            nc.tensor.matmul(
                                ps[:, g, :, :],
                                lhsT=xt_sb[:, ct, tt * 128:(tt + 1) * 128],
                                rhs=wv_sb[:, ct, :],
                                start=(ct == 0), stop=(ct == N_CT - 1),
                            )
                    nc.vector.tensor_add(
                        out=vp_sb[:, b, 2 * p:2 * p + 2, :, 0:64],
                        in0=ps,
                        in1=bv_bc,
                    )

            def emit_scores_exp(s, b, ic):
                """scores + exp for head s, batch b, 512-query block ic.
                Returns the 8 exp tiles ([128 keys, 2 jt x 512 q] each)."""
                e_tiles = []
                for jp in range(8):
                    ps = ps_s_pool.tile([128, 1024], f32, tag="ps_s")
                    for j2 in range(2):
                        jt = jp * 2 + j2
                        nc.tensor.matmul(
                            ps[:, j2 * 512:(j2 + 1) * 512],
                            lhsT=kt_sb[64 * s:64 * (s + 1), b,
                                       jt * 128:(jt + 1) * 128],
                            rhs=qt_sb[64 * s:64 * (s + 1), b,
                                      ic * 512:(ic + 1) * 512],
                            start=True, stop=True,
                        )
                    # constant shift (cancels in the softmax ratio) keeps the
                    # largest exp values small; softmax scale applied here in
                    # f32 rather than folded into quantized weights
                    e = epool.tile([128, 1024], bf16, tag="e")
                    nc.scalar.activation(
                        out=e, in_=ps,
                        func=mybir.ActivationFunctionType.Exp,
                        scale=SCALE, bias=eshift[:, 0:1],
                    )
                    e_tiles.append(e)
                return e_tiles

            def emit_attnv(s, b, ic, e_tiles):
                """attn@V + normalize + transpose into ot_sb."""
                ps_o = ps_o_pool.tile([128, 4, 65], f32, tag="ps_o")
                for qt in range(4):
                    for jt in range(N_JT):
                        e = e_tiles[jt // 2]
                        qoff = (jt % 2) * 512 + qt * 128
                        nc.tensor.matmul(
                            ps_o[:, qt, :],
                            lhsT=e[:, qoff:qoff + 128],
                            rhs=vp_sb[:, b, jt, s, :],
                            start=(jt == 0), stop=(jt == N_JT - 1),
                        )
                r = rpool.tile([128, 4, 1], f32, tag="r")
                nc.vector.reciprocal(out=r, in_=ps_o[:, :, 64:65])
                for qt in range(4):
                    o_t = opool.tile([128, 64], bf16, tag="o")
                    nc.vector.tensor_scalar_mul(
                        out=o_t, in0=ps_o[:, qt, 0:64], scalar1=r[:, qt, :],
                    )
                    tp = misc.tile([64, 128], bf16, tag="mpsum")
                    nc.tensor.transpose(tp, in_=o_t, identity=ident)
                    nc.vector.tensor_copy(
                        out=ot_sb[:, s, b,
                                  ic * 512 + qt * 128:ic * 512 + (qt + 1) * 128],
                        in_=tp,
                    )

            # partial projection accumulator (head-1 od chunks, + bias)
            y_acc = persist.tile([128, 4, C], f32)

            at_in = [dram.tile([512, 512], bf16, name=f"at_in{s}")
                     for s in range(HPC)]
            at_out = [dram.tile([512, 512], bf16, name=f"at_out{s}")
                      for s in range(HPC)]

            def emit_at_slice(s, b, ic):
                """ship one completed (head, batch, token-block) o^T slice
                into the AllToAll staging buffer as soon as it exists, so
                only a 64KB slice DMA precedes the final collective."""
                nc.sync.dma_start(
                    out=at_in[s].rearrange("(r p) t -> p r t", p=64)[
                        :, 4 * b + ic, :],
                    in_=ot_sb[:, s, b, ic * 512:(ic + 1) * 512],
                )

            def emit_at(s):
                """8-way AllToAll of head s's o^T + landing DMA into ogt."""
                nc.gpsimd.collective_compute(
                    "AllToAll",
                    mybir.AluOpType.bypass,
                    ins=[at_in[s][:].opt()],
                    outs=[at_out[s][:].opt()],
                    replica_groups=GROUPS8,
                )
                nc.sync.dma_start(
                    out=ogt_sb[:, s * 4:(s + 1) * 4, :],
                    in_=at_out[s].rearrange("(o p) t -> p o t", p=128),
                )

            def emit_partial_proj(tt):
                """proj over head-1 od chunks (4..7) into y_acc, + bias."""
                for nc2 in range(2):
                    ps = misc.tile([128, 512], f32, tag="mpsum",
                                   name=f"ps_pp{tt}_{nc2}")
                    for i, od in enumerate(range(4, 8)):
                        nc.tensor.matmul(
                            ps,
                            lhsT=ogt_sb[:, od, tt * 128:(tt + 1) * 128],
                            rhs=wp_sb[:, od, nc2 * 512:(nc2 + 1) * 512],
                            start=(i == 0), stop=(i == 3),
                        )
                    nc.vector.tensor_add(
                        out=y_acc[:, tt, nc2 * 512:(nc2 + 1) * 512],
                        in0=ps,
                        in1=bp_bc[:, nc2 * 512:(nc2 + 1) * 512],
                    )

            # ---------------- emission schedule ----------------
            # Unit order: (h0,b0), (h1,b0), (h1,b1), (h0,b1).
            # AT(h1) fires after unit 2 and hides under unit 3; partial
            # projection of the h1 od-chunks runs inside unit 3's blocks;
            # only AT(h0) + the h0 half of proj remain at the end.
            units = [(0, 0), (1, 0), (1, 1), (0, 1)]
            # per-(unit,ic) PE work interleaved into the blocks.  "pre"
            # slots produce data the NEXT block's scores need (q segments)
            # and run before the lookahead scores; "post" slots (v, weight
            # DMAs, ...) run after them, before attnV of the current block.
            # a pre-slot delays the next block's scores+exp, so only the
            # first q segment (needed by the immediately following lookahead)
            # lives there; everything else is emitted post-scores.
            slots_pre = {
                (0, 0): [lambda: emit_qk(0, "q", [1])],
            }
            slots_post = {
                # v(b0) must be fully emitted before unit 0's first attnV.
                # x(b1) load goes after the last emitted x(b0) reader
                # (q(b0) seg3, the (0,2) pre-slot).
                (0, 0): [lambda: emit_qk(0, "q", [2]),
                         lambda: emit_v(0, range(8))],
                (0, 1): [lambda: emit_qk(0, "q", [3])],
                (0, 2): [lambda: emit_xt_dma(1)],
                (0, 3): [lambda: emit_qk(1, "k", [0])],
                (1, 0): [lambda: emit_qk(1, "k", [1]),
                         lambda: nc.sync.dma_start(
                             out=wp_sb,
                             in_=wp_t.rearrange("(od p) c -> p od c", p=128))],
                (1, 1): [lambda: emit_qk(1, "k", [2]),
                         lambda: emit_v(1, [0]),
                         lambda: nc.sync.dma_start(
                             out=bp_bc,
                             in_=bass.AP(tensor=bp.tensor, offset=bp.offset,
                                         ap=[[0, 128]] + bp.ap))],
                (1, 2): [lambda: emit_qk(1, "k", [3]),
                         lambda: emit_qk(1, "q", [0]),
                         lambda: emit_v(1, [1])],
                (1, 3): [lambda: emit_qk(1, "q", [1]),
                         lambda: emit_v(1, [2, 3])],
                (2, 0): [lambda: emit_qk(1, "q", [2]),
                         lambda: emit_v(1, range(4, 8))],
                (2, 1): [lambda: emit_qk(1, "q", [3])],
            }

            # k(b0) all 4 segments + q(b0) seg0 accumulate in five
            # concurrent psum groups, interleaved by contraction chunk, so
            # they all complete as the last x chunk lands (the attention
            # ps_s pool is idle during the fill and lends 4 banks)
            kq_ps = [ps_s_pool.tile([128, 2, 512], f32, tag="ps_s",
                                    name=f"kqps{i}") for i in range(2)]
            q0_ps = misc.tile([128, 512], f32, tag="mpsum", name="q0ps")
            for ct in range(N_CT):
                for seg in range(4):
                    nc.tensor.matmul(
                        kq_ps[seg // 2][:, seg % 2, :],
                        lhsT=wk_sb[:, ct, :],
                        rhs=xt_sb[:, ct, seg * 512:(seg + 1) * 512],
                        start=(ct == 0), stop=(ct == N_CT - 1),
                    )
                nc.tensor.matmul(
                    q0_ps,
                    lhsT=wq_sb[:, ct, :],
                    rhs=xt_sb[:, ct, 0:512],
                    start=(ct == 0), stop=(ct == N_CT - 1),
                )
            for seg in range(4):
                nc.vector.tensor_scalar_add(
                    out=kt_sb[:, 0, seg * 512:(seg + 1) * 512],
                    in0=kq_ps[seg // 2][:, seg % 2, :],
                    scalar1=bqk_sb[:, 1:2],
                )
            nc.vector.tensor_scalar_add(
                out=qt_sb[:, 0, 0:512], in0=q0_ps,
                scalar1=bqk_sb[:, 0:1],
            )

            # flat block list, software-pipelined one block ahead: scores+exp
            # for block j+1 are emitted before attnV of block j, so the
            # activation engine never waits out attnV/transposes at block and
            # unit boundaries.
            blocks = [(ui, s, b, ic)
                      for ui, (s, b) in enumerate(units)
                      for ic in range(N_IC)]
            e_cur = emit_scores_exp(*blocks[0][1:])
            for j, (ui, s, b, ic) in enumerate(blocks):
                for work in slots_pre.get((ui, ic), []):
                    work()
                if j + 1 < len(blocks):
                    e_next = emit_scores_exp(*blocks[j + 1][1:])
                for work in slots_post.get((ui, ic), []):
                    work()
                emit_attnv(s, b, ic, e_cur)
                emit_at_slice(s, b, ic)
                e_cur = e_next
                if ic == N_IC - 1:
                    if ui == 2:
                        emit_at(1)
                    elif ui == 3:
                        emit_at(0)

            # partial projection over the h1 od chunks (landed with AT(1))
            # overlaps AT(0)'s transfer
            for tt in range(TOKS // 128):
                emit_partial_proj(tt)

            # PE warmer: keep the tensor engine clocked up through the AT(0)
            # wait so the final projection issues at full p-state instead of
            # restarting from the low-clock ramp after ~25us of idle.
            warm = misc.tile([128, 512], f32, tag="mpsum", name="warm")
            for _ in range(96):
                nc.tensor.matmul(warm, lhsT=ogt_sb[:, 4, 0:128],
                                 rhs=wp_sb[:, 4, 0:512],
                                 start=True, stop=True)

            # ---------------- final projection (head-0 od chunks) ----------
            # runs after AT(0) lands; uses the attention-phase ps_s pool
            # (free by now) so each token tile needs one psum tile and one
            # wide DVE add
            for tt in range(TOKS // 128):
                ps_y = ps_s_pool.tile([128, 1024], f32, tag="ps_s",
                                      name=f"ps_y{tt}")
                for nc2 in range(2):
                    for od in range(4):
                        nc.tensor.matmul(
                            ps_y[:, nc2 * 512:(nc2 + 1) * 512],
                            lhsT=ogt_sb[:, od, tt * 128:(tt + 1) * 128],
                            rhs=wp_sb[:, od, nc2 * 512:(nc2 + 1) * 512],
                            start=(od == 0), stop=(od == 3),
                        )
                y_sb = ypool.tile([128, C], f32, tag="y_sb")
                nc.vector.tensor_add(
                    out=y_sb, in0=ps_y, in1=y_acc[:, tt, :],
                )
                nc.sync.dma_start(out=y[tt * 128:(tt + 1) * 128, :], in_=y_sb)

    nc.compile()
    return nc


_CACHE = {}


def _get_nc():
    if "nc" not in _CACHE:
        _CACHE["nc"] = build_kernel()
    return _CACHE["nc"]


def make_in_maps(x, W_qkv, b_qkv, W_proj, b_proj):
    x = np.asarray(x, dtype=np.float32)
    W_qkv = np.asarray(W_qkv, dtype=np.float32)
    b_qkv = np.asarray(b_qkv, dtype=np.float32)
    W_proj = np.asarray(W_proj, dtype=np.float32)
    b_proj = np.asarray(b_proj, dtype=np.float32)

    Wq = W_qkv[0:C]
    Wk = W_qkv[C:2 * C]
    Wv = W_qkv[2 * C:3 * C]
    bq = b_qkv[0:C]
    bk = b_qkv[C:2 * C]
    bv_full = b_qkv[2 * C:3 * C]

    # x^T for both batches, shared by all cores
    xtb = np.ascontiguousarray(
        x.transpose(0, 2, 1)).astype(np_bf16)  # [B, C, N]

    # W_proj.T with rows permuted to the AllToAll arrival order:
    # od chunk (s, o) holds heads {4o+s, 4o+2+s}
    perm = []
    for s in range(HPC):
        for o in range(4):
            for h in (4 * o + s, 4 * o + 2 + s):
                perm.extend(range(64 * h, 64 * (h + 1)))
    wp_t_full = np.ascontiguousarray(W_proj.T[perm, :]).astype(np_bf16)

    in_maps = []
    for core in range(NCORES):
        rows = slice(128 * core, 128 * (core + 1))  # dims of heads 2c, 2c+1
        # fused [k|q|v] weight slab in [ct, 128, 384] layout
        slab = np.concatenate(
            [Wk[rows].T, Wq[rows].T, Wv[rows].T], axis=1)  # [C, 384]
        slab = np.ascontiguousarray(
            slab.reshape(N_CT, 128, 3 * 128)).astype(np_bf16)
        in_maps.append({
            "xt": xtb,
            "wkqv": slab,
            "bqk": np.ascontiguousarray(
                np.stack([bq[rows], bk[rows]], axis=1)),
            "bv": bv_full[rows].astype(np_bf16),
            "wp_t": wp_t_full,
            "bp": b_proj,
        })
    return in_maps


def kernel(x, W_qkv, b_qkv, W_proj, b_proj):
    nc = _get_nc()
    in_maps = make_in_maps(x, W_qkv, b_qkv, W_proj, b_proj)
    res = run_bass_kernel_spmd(nc, in_maps, list(range(NCORES)))

    out = np.empty((B, N, C), dtype=np.float32)
    for core in range(NCORES):
        b = core // 4
        g = core % 4
        out[b, g * TOKS:(g + 1) * TOKS, :] = res.results[core]["y"]
    return out


# revision 27
# speedup vs baseline: 1.4856x; 1.0112x over previous
"""Trainium2 Bass kernel for nn_MultiHeadAttention (dense transformer block).

Reference computation (B=2 batches, N=2048 tokens, C=1024, H=16 heads, D=64):
    qkv  = x @ W_qkv.T + b_qkv
    q,k,v split into heads; attn = softmax(q @ k.T / sqrt(D)); o = attn @ v
    out  = o @ W_proj.T + b_proj

Sharding over 8 NeuronCores: head-parallel attention, token-parallel
projection.  Core c owns heads {2c, 2c+1} and computes QKV + attention for
both batches for those heads.  The per-head outputs o^T are exchanged with a
single 8-way AllToAll per head (each core sends, for every peer r, its head's
o^T slice for peer r's (batch, token-slice)); afterwards each core holds
o^T of ALL 16 heads for its own 512-token slice (batch c//4, tokens
(c%4)*512..) and computes the full output projection there.

All matmuls run in bf16 (fp32 PSUM accumulation).  attn@V uses the exp tile
as the stationary operand and V (with an appended ones column) as the moving
operand, so each matmul streams only 65 rows at full 128x128 PE utilization
and the softmax denominator lands on the same PSUM partition as the outputs
(normalization = per-partition scalar multiply on the vector engine).
"""

import sys

sys.path.insert(0, "/opt/trn_rl_repo")

import numpy as np
import ml_dtypes
import concourse.bass as bass
import concourse.tile as tile
from concourse import mybir, bacc
from concourse.bass_utils import run_bass_kernel_spmd

f32 = mybir.dt.float32
bf16 = mybir.dt.bfloat16
f8 = mybir.dt.float8e4
np_bf16 = ml_dtypes.bfloat16

# problem constants (hardcoded per contract)
B = 2
N = 2048
C = 1024
H = 16
D = C // H  # 64
SCALE = D ** -0.5

NCORES = 8
GROUPS8 = [[0, 1, 2, 3, 4, 5, 6, 7]]
HPC = H // NCORES          # heads per core = 2
TOKS = N // 4              # per-core output token slice = 512
N_CT = C // 128            # contraction chunks over C = 8
N_JT = N // 128            # key tiles = 16
N_IC = N // 512            # query blocks = 4
N_OD = C // 128            # o-dim contraction chunks in proj = 8


def build_kernel():
    nc = bacc.Bacc("TRN2", target_bir_lowering=False, debug=False,
                   num_devices=NCORES)

    # ---- DRAM I/O (all bf16 except biases / final output) ----
    xt = nc.dram_tensor("xt", [B, C, N], bf16, kind="ExternalInput").ap()
    # fused [k|q|v] weight slab: [ct, 128 part, 3*128] -> single DMA
    wkqv = nc.dram_tensor("wkqv", [N_CT, 128, 3 * 128], bf16,
                          kind="ExternalInput").ap()
    bqk = nc.dram_tensor("bqk", [128, 2], f32, kind="ExternalInput").ap()
    bv = nc.dram_tensor("bv", [128], bf16, kind="ExternalInput").ap()
    wp_t = nc.dram_tensor("wp_t", [C, C], bf16, kind="ExternalInput").ap()
    bp = nc.dram_tensor("bp", [C], f32, kind="ExternalInput").ap()
    y = nc.dram_tensor("y", [TOKS, C], f32, kind="ExternalOutput").ap()

    with tile.TileContext(nc, pool_alloc_mode="queue") as tc:
        with (
            tc.tile_pool(name="consts", bufs=1) as consts,
            tc.tile_pool(name="persist", bufs=1) as persist,
            tc.tile_pool(name="p1x", bufs=1) as p1x,
            tc.tile_pool(name="p1w", bufs=1) as p1w,
            tc.tile_pool(name="epool", bufs=18) as epool,
            tc.tile_pool(name="opool", bufs=6) as opool,
            tc.tile_pool(name="rpool", bufs=4) as rpool,
            tc.tile_pool(name="ypool", bufs=3) as ypool,
            tc.tile_pool(name="ps_s", bufs=2, space="PSUM") as ps_s_pool,
            tc.tile_pool(name="ps_o", bufs=1, space="PSUM") as ps_o_pool,
            tc.tile_pool(name="misc", bufs=3, space="PSUM") as misc,
            tc.tile_pool(name="dram", bufs=1, space="DRAM") as dram,
        ):
            # ---------------- constants ----------------
            bqk_sb = consts.tile([128, 2], f32)
            nc.sync.dma_start(out=bqk_sb, in_=bqk)
            # V bias broadcast: [128 part, jt-dup 2, h 2, d 64]
            bv_bc = consts.tile([128, 2, 2, 64], bf16)
            nc.sync.dma_start(
                out=bv_bc,
                in_=bass.AP(tensor=bv.tensor, offset=bv.offset,
                            ap=[[0, 128], [0, 2], [64, 2], [1, 64]]),
            )
            bp_bc = consts.tile([128, C], f32)
            # exp shift (keeps fp8 exp in range; cancels in softmax ratio)
            eshift = consts.tile([128, 1], f32)
            nc.vector.memset(eshift, -4.0)
            # 128x128 bf16 identity (moving operand of PE transposes)
            ident = consts.tile([128, 128], bf16)
            nc.gpsimd.memset(ident, 0.0)
            nc.gpsimd.affine_select(
                out=ident, in_=ident,
                compare_op=mybir.AluOpType.not_equal,
                fill=1.0, base=0, pattern=[[-1, 128]], channel_multiplier=1,
            )

            # -------------- persistent activations --------------
            # partition dim = 2 local heads x 64 dims (bf16: fp8 attention
            # was tested and fails the 2e-2 tolerance - the near-diagonal
            # logits reach 9.0, so rows are peaked and quantization noise
            # on q/k/e/v transfers directly into the output)
            qt_sb = persist.tile([128, B, N], bf16)   # q^T
            kt_sb = persist.tile([128, B, N], bf16)   # k^T
            # V natural + ones column: [tok-part, b, jt, h, 65]
            vp_sb = persist.tile([128, B, N_JT, HPC, 65], bf16)
            nc.vector.memset(vp_sb[:, :, :, :, 64:65], 1.0)
            # o^T (normalized): [64 dims, head, b, t] - 64-partition tile so
            # all engine copies into it stay partition-base aligned
            ot_sb = persist.tile([64, HPC, B, N], bf16)

            # weight / x staging
            xt_sb = p1x.tile([128, N_CT, N], bf16)
            wkqv_sb = p1w.tile([128, N_CT, 3 * 128], bf16)
            wk_sb = wkqv_sb[:, :, 0:128]
            wq_sb = wkqv_sb[:, :, 128:256]
            wv_sb = wkqv_sb[:, :, 256:384]
            wp_sb = p1w.tile([128, N_OD, C], bf16)
            ogt_sb = persist.tile([128, N_OD, TOKS], bf16)

            xt_views = [
                xt[b].rearrange("(ct p) n -> p ct n", p=128) for b in range(B)
            ]

            def emit_xt_dma(b):
                for ct in range(N_CT):
                    nc.sync.dma_start(out=xt_sb[:, ct, :],
                                      in_=xt_views[b][:, ct, :])

            # W_kqv (one instruction) + x^T(b0); W_p deferred until the
            # attention phase is underway (it is first read ~150us in).
            nc.sync.dma_start(out=wkqv_sb,
                              in_=wkqv.rearrange("ct p m -> p ct m"))
            emit_xt_dma(0)

            def emit_qk(b, tensor, segs):
                """q^T / k^T projection for 512-token segments `segs`."""
                w_sb = wq_sb if tensor == "q" else wk_sb
                bcol = 0 if tensor == "q" else 1
                for seg in segs:
                    ps = misc.tile([128, 512], f32, tag="mpsum")
                    for ct in range(N_CT):
                        nc.tensor.matmul(
                            ps,
                            lhsT=w_sb[:, ct, :],
                            rhs=xt_sb[:, ct, seg * 512:(seg + 1) * 512],
                            start=(ct == 0), stop=(ct == N_CT - 1),
                        )
                    sl = slice(seg * 512, (seg + 1) * 512)
                    dst = qt_sb[:, b, sl] if tensor == "q" else kt_sb[:, b, sl]
                    nc.vector.tensor_scalar_add(
                        out=dst,
                        in0=ps,
                        scalar1=bqk_sb[:, bcol:bcol + 1],
                    )

            def emit_v(b, pairs):
                """V natural projection for pairs of 128-token tiles."""
                for p in pairs:
                    ps = misc.tile([128, 2, HPC, 64], f32, tag="mpsum")
                    for g in range(2):
                        tt = 2 * p + g
                        for ct in range(N_CT):
                            nc.tensor.matmul(
                                ps[:, g, :, :],
                                lhsT=xt_sb[:, ct, tt * 128:(tt + 1) * 128],
                                rhs=wv_sb[:, ct, :],
                                start=(ct == 0), stop=(ct == N_CT - 1),
                            )
                    nc.vector.tensor_add(
                        out=vp_sb[:, b, 2 * p:2 * p + 2, :, 0:64],
                        in0=ps,
                        in1=bv_bc,
                    )

            def emit_scores_exp(s, b, ic):
                """scores + exp for head s, batch b, 512-query block ic.
                Returns the 8 exp tiles ([128 keys, 2 jt x 512 q] each)."""
                e_tiles = []
                for jp in range(8):
                    ps = ps_s_pool.tile([128, 1024], f32, tag="ps_s")
                    for j2 in range(2):
                        jt = jp * 2 + j2
                        nc.tensor.matmul(
                            ps[:, j2 * 512:(j2 + 1) * 512],
                            lhsT=kt_sb[64 * s:64 * (s + 1), b,
                                       jt * 128:(jt + 1) * 128],
                            rhs=qt_sb[64 * s:64 * (s + 1), b,
                                      ic * 512:(ic + 1) * 512],
                            start=True, stop=True,
                        )
                    # constant shift (cancels in the softmax ratio) keeps the
                    # largest exp values small; softmax scale applied here in
                    # f32 rather than folded into quantized weights
                    e = epool.tile([128, 1024], bf16, tag="e")
                    nc.scalar.activation(
                        out=e, in_=ps,
                        func=mybir.ActivationFunctionType.Exp,
                        scale=SCALE, bias=eshift[:, 0:1],
                    )
                    e_tiles.append(e)
                return e_tiles

            def emit_attnv(s, b, ic, e_tiles, last=False):
                """attn@V + normalize + transpose into ot_sb.  For the
                final pre-collective block the normalize multiplies run on
                the (by then idle) activation engine, shortening the
                cross-engine latency chain ahead of the exposed AllToAll."""
                ps_o = ps_o_pool.tile([128, 4, 65], f32, tag="ps_o")
                for qt in range(4):
                    for jt in range(N_JT):
                        e = e_tiles[jt // 2]
                        qoff = (jt % 2) * 512 + qt * 128
                        nc.tensor.matmul(
                            ps_o[:, qt, :],
                            lhsT=e[:, qoff:qoff + 128],
                            rhs=vp_sb[:, b, jt, s, :],
                            start=(jt == 0), stop=(jt == N_JT - 1),
                        )
                r = rpool.tile([128, 4, 1], f32, tag="r")
                nc.vector.reciprocal(out=r, in_=ps_o[:, :, 64:65])
                for qt in range(4):
                    o_t = opool.tile([128, 64], bf16, tag="o")
                    if last:
                        nc.scalar.activation(
                            out=o_t, in_=ps_o[:, qt, 0:64],
                            func=mybir.ActivationFunctionType.Copy,
                            scale=r[:, qt, :],
                        )
                    else:
                        nc.vector.tensor_scalar_mul(
                            out=o_t, in0=ps_o[:, qt, 0:64], scalar1=r[:, qt, :],
                        )
                    tp = misc.tile([64, 128], bf16, tag="mpsum")
                    nc.tensor.transpose(tp, in_=o_t, identity=ident)
                    nc.vector.tensor_copy(
                        out=ot_sb[:, s, b,
                                  ic * 512 + qt * 128:ic * 512 + (qt + 1) * 128],
                        in_=tp,
                    )

            # partial projection accumulator (head-1 od chunks, + bias)
            y_acc = persist.tile([128, 4, C], f32)

            at_in = [dram.tile([512, 512], bf16, name=f"at_in{s}")
                     for s in range(HPC)]
            at_out = [dram.tile([512, 512], bf16, name=f"at_out{s}")
                      for s in range(HPC)]

            def emit_at_slice(s, b, ic):
                """ship one completed (head, batch, token-block) o^T slice
                into the AllToAll staging buffer as soon as it exists, so
                only a 64KB slice DMA precedes the final collective."""
                nc.sync.dma_start(
                    out=at_in[s].rearrange("(r p) t -> p r t", p=64)[
                        :, 4 * b + ic, :],
                    in_=ot_sb[:, s, b, ic * 512:(ic + 1) * 512],
                )

            def emit_at(s):
                """8-way AllToAll of head s's o^T + landing DMA into ogt.
                Head 0's landing is chunked per od so the final projection's
                first accumulation chunk starts as early as possible."""
                nc.gpsimd.collective_compute(
                    "AllToAll",
                    mybir.AluOpType.bypass,
                    ins=[at_in[s][:].opt()],
                    outs=[at_out[s][:].opt()],
                    replica_groups=GROUPS8,
                )
                view = at_out[s].rearrange("(o p) t -> p o t", p=128)
                if s == 0:
                    for o in range(4):
                        nc.sync.dma_start(out=ogt_sb[:, o, :],
                                          in_=view[:, o, :])
                else:
                    nc.sync.dma_start(
                        out=ogt_sb[:, s * 4:(s + 1) * 4, :], in_=view)

            def emit_partial_proj(tt):
                """proj over head-1 od chunks (4..7) into y_acc, + bias."""
                for nc2 in range(2):
                    ps = misc.tile([128, 512], f32, tag="mpsum",
                                   name=f"ps_pp{tt}_{nc2}")
                    for i, od in enumerate(range(4, 8)):
                        nc.tensor.matmul(
                            ps,
                            lhsT=ogt_sb[:, od, tt * 128:(tt + 1) * 128],
                            rhs=wp_sb[:, od, nc2 * 512:(nc2 + 1) * 512],
                            start=(i == 0), stop=(i == 3),
                        )
                    nc.vector.tensor_add(
                        out=y_acc[:, tt, nc2 * 512:(nc2 + 1) * 512],
                        in0=ps,
                        in1=bp_bc[:, nc2 * 512:(nc2 + 1) * 512],
                    )

            # ---------------- emission schedule ----------------
            # Unit order: (h0,b0), (h1,b0), (h1,b1), (h0,b1).
            # AT(h1) fires after unit 2 and hides under unit 3; partial
            # projection of the h1 od-chunks runs inside unit 3's blocks;
            # only AT(h0) + the h0 half of proj remain at the end.
            units = [(0, 0), (1, 0), (1, 1), (0, 1)]
            # per-(unit,ic) PE work interleaved into the blocks.  "pre"
            # slots produce data the NEXT block's scores need (q segments)
            # and run before the lookahead scores; "post" slots (v, weight
            # DMAs, ...) run after them, before attnV of the current block.
            # a pre-slot delays the next block's scores+exp, so only the
            # first q segment (needed by the immediately following lookahead)
            # lives there; everything else is emitted post-scores.
            slots_pre = {
                (0, 0): [lambda: emit_qk(0, "q", [1])],
            }
            slots_post = {
                # v(b0) must be fully emitted before unit 0's first attnV.
                # x(b1) load goes after the last emitted x(b0) reader
                # (q(b0) seg3, the (0,2) pre-slot).
                (0, 0): [lambda: emit_qk(0, "q", [2]),
                         lambda: emit_v(0, range(8))],
                (0, 1): [lambda: emit_qk(0, "q", [3])],
                (0, 2): [lambda: emit_xt_dma(1)],
                (0, 3): [lambda: emit_qk(1, "k", [0])],
                (1, 0): [lambda: emit_qk(1, "k", [1]),
                         lambda: nc.sync.dma_start(
                             out=wp_sb,
                             in_=wp_t.rearrange("(od p) c -> p od c", p=128))],
                (1, 1): [lambda: emit_qk(1, "k", [2]),
                         lambda: emit_v(1, [0]),
                         lambda: nc.sync.dma_start(
                             out=bp_bc,
                             in_=bass.AP(tensor=bp.tensor, offset=bp.offset,
                                         ap=[[0, 128]] + bp.ap))],
                (1, 2): [lambda: emit_qk(1, "k", [3]),
                         lambda: emit_qk(1, "q", [0]),
                         lambda: emit_v(1, [1])],
                (1, 3): [lambda: emit_qk(1, "q", [1]),
                         lambda: emit_v(1, [2, 3])],
                (2, 0): [lambda: emit_qk(1, "q", [2]),
                         lambda: emit_v(1, range(4, 8))],
                (2, 1): [lambda: emit_qk(1, "q", [3])],
            }

            emit_qk(0, "k", range(4))
            emit_qk(0, "q", [0])

            # flat block list, software-pipelined one block ahead: scores+exp
            # for block j+1 are emitted before attnV of block j, so the
            # activation engine never waits out attnV/transposes at block and
            # unit boundaries.
            blocks = [(ui, s, b, ic)
                      for ui, (s, b) in enumerate(units)
                      for ic in range(N_IC)]
            e_cur = emit_scores_exp(*blocks[0][1:])
            for j, (ui, s, b, ic) in enumerate(blocks):
                for work in slots_pre.get((ui, ic), []):
                    work()
                if j + 1 < len(blocks):
                    e_next = emit_scores_exp(*blocks[j + 1][1:])
                for work in slots_post.get((ui, ic), []):
                    work()
                emit_attnv(s, b, ic, e_cur, last=(j == len(blocks) - 1))
                emit_at_slice(s, b, ic)
                e_cur = e_next
                if ic == N_IC - 1:
                    if ui == 2:
                        emit_at(1)
                    elif ui == 3:
                        emit_at(0)

            # partial projection over the h1 od chunks (landed with AT(1))
            # overlaps AT(0)'s transfer
            for tt in range(TOKS // 128):
                emit_partial_proj(tt)

            # PE warmer: keep the tensor engine clocked up through the AT(0)
            # wait so the final projection issues at full p-state instead of
            # restarting from the low-clock ramp after ~25us of idle.
            warm = misc.tile([128, 512], f32, tag="mpsum", name="warm")
            for _ in range(76):
                nc.tensor.matmul(warm, lhsT=ogt_sb[:, 4, 0:128],
                                 rhs=wp_sb[:, 4, 0:512],
                                 start=True, stop=True)

            # ---------------- final projection (head-0 od chunks) ----------
            # runs after AT(0) lands; uses the attention-phase ps_s pool
            # (free by now) so each token tile needs one psum tile and one
            # wide DVE add
            for tt in range(TOKS // 128):
                ps_y = ps_s_pool.tile([128, 1024], f32, tag="ps_s",
                                      name=f"ps_y{tt}")
                for nc2 in range(2):
                    for od in range(4):
                        nc.tensor.matmul(
                            ps_y[:, nc2 * 512:(nc2 + 1) * 512],
                            lhsT=ogt_sb[:, od, tt * 128:(tt + 1) * 128],
                            rhs=wp_sb[:, od, nc2 * 512:(nc2 + 1) * 512],
                            start=(od == 0), stop=(od == 3),
                        )
                y_sb = ypool.tile([128, C], f32, tag="y_sb")
                nc.vector.tensor_add(
                    out=y_sb, in0=ps_y, in1=y_acc[:, tt, :],
                )
                nc.sync.dma_start(out=y[tt * 128:(tt + 1) * 128, :], in_=y_sb)

    nc.compile()
    return nc


_CACHE = {}


def _get_nc():
    if "nc" not in _CACHE:
        _CACHE["nc"] = build_kernel()
    return _CACHE["nc"]


def make_in_maps(x, W_qkv, b_qkv, W_proj, b_proj):
    x = np.asarray(x, dtype=np.float32)
    W_qkv = np.asarray(W_qkv, dtype=np.float32)
    b_qkv = np.asarray(b_qkv, dtype=np.float32)
    W_proj = np.asarray(W_proj, dtype=np.float32)
    b_proj = np.asarray(b_proj, dtype=np.float32)

    Wq = W_qkv[0:C]
    Wk = W_qkv[C:2 * C]
    Wv = W_qkv[2 * C:3 * C]
    bq = b_qkv[0:C]
    bk = b_qkv[C:2 * C]
    bv_full = b_qkv[2 * C:3 * C]

    # x^T for both batches, shared by all cores
    xtb = np.ascontiguousarray(
        x.transpose(0, 2, 1)).astype(np_bf16)  # [B, C, N]

    # W_proj.T with rows permuted to the AllToAll arrival order:
    # od chunk (s, o) holds heads {4o+s, 4o+2+s}
    perm = []
    for s in range(HPC):
        for o in range(4):
            for h in (4 * o + s, 4 * o + 2 + s):
                perm.extend(range(64 * h, 64 * (h + 1)))
    wp_t_full = np.ascontiguousarray(W_proj.T[perm, :]).astype(np_bf16)

    in_maps = []
    for core in range(NCORES):
        rows = slice(128 * core, 128 * (core + 1))  # dims of heads 2c, 2c+1
        # fused [k|q|v] weight slab in [ct, 128, 384] layout
        slab = np.concatenate(
            [Wk[rows].T, Wq[rows].T, Wv[rows].T], axis=1)  # [C, 384]
        slab = np.ascontiguousarray(
            slab.reshape(N_CT, 128, 3 * 128)).astype(np_bf16)
        in_maps.append({
            "xt": xtb,
            "wkqv": slab,
            "bqk": np.ascontiguousarray(
                np.stack([bq[rows], bk[rows]], axis=1)),
            "bv": bv_full[rows].astype(np_bf16),
            "wp_t": wp_t_full,
            "bp": b_proj,
        })
    return in_maps


def kernel(x, W_qkv, b_qkv, W_proj, b_proj):
    nc = _get_nc()
    in_maps = make_in_maps(x, W_qkv, b_qkv, W_proj, b_proj)
    res = run_bass_kernel_spmd(nc, in_maps, list(range(NCORES)))

    out = np.empty((B, N, C), dtype=np.float32)
    for core in range(NCORES):
        b = core // 4
        g = core % 4
        out[b, g * TOKS:(g + 1) * TOKS, :] = res.results[core]["y"]
    return out


# revision 29
# speedup vs baseline: 1.4906x; 1.0034x over previous
"""Trainium2 Bass kernel for nn_MultiHeadAttention (dense transformer block).

Reference computation (B=2 batches, N=2048 tokens, C=1024, H=16 heads, D=64):
    qkv  = x @ W_qkv.T + b_qkv
    q,k,v split into heads; attn = softmax(q @ k.T / sqrt(D)); o = attn @ v
    out  = o @ W_proj.T + b_proj

Sharding over 8 NeuronCores: head-parallel attention, token-parallel
projection.  Core c owns heads {2c, 2c+1} and computes QKV + attention for
both batches for those heads.  The per-head outputs o^T are exchanged with a
single 8-way AllToAll per head (each core sends, for every peer r, its head's
o^T slice for peer r's (batch, token-slice)); afterwards each core holds
o^T of ALL 16 heads for its own 512-token slice (batch c//4, tokens
(c%4)*512..) and computes the full output projection there.

All matmuls run in bf16 (fp32 PSUM accumulation).  attn@V uses the exp tile
as the stationary operand and V (with an appended ones column) as the moving
operand, so each matmul streams only 65 rows at full 128x128 PE utilization
and the softmax denominator lands on the same PSUM partition as the outputs
(normalization = per-partition scalar multiply on the vector engine).
"""

import sys

sys.path.insert(0, "/opt/trn_rl_repo")

import numpy as np
import ml_dtypes
import concourse.bass as bass
import concourse.tile as tile
from concourse import mybir, bacc
from concourse.bass_utils import run_bass_kernel_spmd

f32 = mybir.dt.float32
bf16 = mybir.dt.bfloat16
f8 = mybir.dt.float8e4
np_bf16 = ml_dtypes.bfloat16

# problem constants (hardcoded per contract)
B = 2
N = 2048
C = 1024
H = 16
D = C // H  # 64
SCALE = D ** -0.5

NCORES = 8
GROUPS8 = [[0, 1, 2, 3, 4, 5, 6, 7]]
HPC = H // NCORES          # heads per core = 2
TOKS = N // 4              # per-core output token slice = 512
N_CT = C // 128            # contraction chunks over C = 8
N_JT = N // 128            # key tiles = 16
N_IC = N // 512            # query blocks = 4
N_OD = C // 128            # o-dim contraction chunks in proj = 8


def build_kernel():
    nc = bacc.Bacc("TRN2", target_bir_lowering=False, debug=False,
                   num_devices=NCORES)

    # ---- DRAM I/O (all bf16 except biases / final output) ----
    xt = nc.dram_tensor("xt", [B, C, N], bf16, kind="ExternalInput").ap()
    # fused [k|q|v] weight slab: [ct, 128 part, 3*128] -> single DMA
    wkqv = nc.dram_tensor("wkqv", [N_CT, 128, 3 * 128], bf16,
                          kind="ExternalInput").ap()
    bqk = nc.dram_tensor("bqk", [128, 2], f32, kind="ExternalInput").ap()
    bv = nc.dram_tensor("bv", [128], bf16, kind="ExternalInput").ap()
    wp_t = nc.dram_tensor("wp_t", [C, C], bf16, kind="ExternalInput").ap()
    bp = nc.dram_tensor("bp", [C], f32, kind="ExternalInput").ap()
    y = nc.dram_tensor("y", [TOKS, C], bf16, kind="ExternalOutput").ap()

    with tile.TileContext(nc, pool_alloc_mode="queue") as tc:
        with (
            tc.tile_pool(name="consts", bufs=1) as consts,
            tc.tile_pool(name="persist", bufs=1) as persist,
            tc.tile_pool(name="p1x", bufs=1) as p1x,
            tc.tile_pool(name="p1w", bufs=1) as p1w,
            tc.tile_pool(name="epool", bufs=18) as epool,
            tc.tile_pool(name="opool", bufs=6) as opool,
            tc.tile_pool(name="rpool", bufs=4) as rpool,
            tc.tile_pool(name="ypool", bufs=3) as ypool,
            tc.tile_pool(name="ps_s", bufs=2, space="PSUM") as ps_s_pool,
            tc.tile_pool(name="ps_o", bufs=1, space="PSUM") as ps_o_pool,
            tc.tile_pool(name="misc", bufs=3, space="PSUM") as misc,
            tc.tile_pool(name="dram", bufs=1, space="DRAM") as dram,
        ):
            # ---------------- constants ----------------
            bqk_sb = consts.tile([128, 2], f32)
            nc.sync.dma_start(out=bqk_sb, in_=bqk)
            # V bias broadcast: [128 part, jt-dup 2, h 2, d 64]
            bv_bc = consts.tile([128, 2, 2, 64], bf16)
            nc.sync.dma_start(
                out=bv_bc,
                in_=bass.AP(tensor=bv.tensor, offset=bv.offset,
                            ap=[[0, 128], [0, 2], [64, 2], [1, 64]]),
            )
            bp_bc = consts.tile([128, C], f32)
            # exp shift (keeps fp8 exp in range; cancels in softmax ratio)
            eshift = consts.tile([128, 1], f32)
            nc.vector.memset(eshift, -4.0)
            # 128x128 bf16 identity (moving operand of PE transposes)
            ident = consts.tile([128, 128], bf16)
            nc.gpsimd.memset(ident, 0.0)
            nc.gpsimd.affine_select(
                out=ident, in_=ident,
                compare_op=mybir.AluOpType.not_equal,
                fill=1.0, base=0, pattern=[[-1, 128]], channel_multiplier=1,
            )

            # -------------- persistent activations --------------
            # partition dim = 2 local heads x 64 dims (bf16: fp8 attention
            # was tested and fails the 2e-2 tolerance - the near-diagonal
            # logits reach 9.0, so rows are peaked and quantization noise
            # on q/k/e/v transfers directly into the output)
            qt_sb = persist.tile([128, B, N], bf16)   # q^T
            kt_sb = persist.tile([128, B, N], bf16)   # k^T
            # V natural + ones column: [tok-part, b, jt, h, 65]
            vp_sb = persist.tile([128, B, N_JT, HPC, 65], bf16)
            nc.vector.memset(vp_sb[:, :, :, :, 64:65], 1.0)
            # o^T (normalized): [64 dims, head, b, t] - 64-partition tile so
            # all engine copies into it stay partition-base aligned
            ot_sb = persist.tile([64, HPC, B, N], bf16)

            # weight / x staging
            xt_sb = p1x.tile([128, N_CT, N], bf16)
            wkqv_sb = p1w.tile([128, N_CT, 3 * 128], bf16)
            wk_sb = wkqv_sb[:, :, 0:128]
            wq_sb = wkqv_sb[:, :, 128:256]
            wv_sb = wkqv_sb[:, :, 256:384]
            wp_sb = p1w.tile([128, N_OD, C], bf16)
            ogt_sb = persist.tile([128, N_OD, TOKS], bf16)

            xt_views = [
                xt[b].rearrange("(ct p) n -> p ct n", p=128) for b in range(B)
            ]

            def emit_xt_dma(b):
                for ct in range(N_CT):
                    nc.sync.dma_start(out=xt_sb[:, ct, :],
                                      in_=xt_views[b][:, ct, :])

            # W_kqv (one instruction) + x^T(b0); W_p deferred until the
            # attention phase is underway (it is first read ~150us in).
            nc.sync.dma_start(out=wkqv_sb,
                              in_=wkqv.rearrange("ct p m -> p ct m"))
            emit_xt_dma(0)

            def emit_qk(b, tensor, segs):
                """q^T / k^T projection for 512-token segments `segs`."""
                w_sb = wq_sb if tensor == "q" else wk_sb
                bcol = 0 if tensor == "q" else 1
                for seg in segs:
                    ps = misc.tile([128, 512], f32, tag="mpsum")
                    for ct in range(N_CT):
                        nc.tensor.matmul(
                            ps,
                            lhsT=w_sb[:, ct, :],
                            rhs=xt_sb[:, ct, seg * 512:(seg + 1) * 512],
                            start=(ct == 0), stop=(ct == N_CT - 1),
                        )
                    sl = slice(seg * 512, (seg + 1) * 512)
                    dst = qt_sb[:, b, sl] if tensor == "q" else kt_sb[:, b, sl]
                    nc.vector.tensor_scalar_add(
                        out=dst,
                        in0=ps,
                        scalar1=bqk_sb[:, bcol:bcol + 1],
                    )

            def emit_v(b, pairs):
                """V natural projection for pairs of 128-token tiles."""
                for p in pairs:
                    ps = misc.tile([128, 2, HPC, 64], f32, tag="mpsum")
                    for g in range(2):
                        tt = 2 * p + g
                        for ct in range(N_CT):
                            nc.tensor.matmul(
                                ps[:, g, :, :],
                                lhsT=xt_sb[:, ct, tt * 128:(tt + 1) * 128],
                                rhs=wv_sb[:, ct, :],
                                start=(ct == 0), stop=(ct == N_CT - 1),
                            )
                    nc.vector.tensor_add(
                        out=vp_sb[:, b, 2 * p:2 * p + 2, :, 0:64],
                        in0=ps,
                        in1=bv_bc,
                    )

            def emit_scores_exp(s, b, ic):
                """scores + exp for head s, batch b, 512-query block ic.
                Returns the 8 exp tiles ([128 keys, 2 jt x 512 q] each)."""
                e_tiles = []
                for jp in range(8):
                    ps = ps_s_pool.tile([128, 1024], f32, tag="ps_s")
                    for j2 in range(2):
                        jt = jp * 2 + j2
                        nc.tensor.matmul(
                            ps[:, j2 * 512:(j2 + 1) * 512],
                            lhsT=kt_sb[64 * s:64 * (s + 1), b,
                                       jt * 128:(jt + 1) * 128],
                            rhs=qt_sb[64 * s:64 * (s + 1), b,
                                      ic * 512:(ic + 1) * 512],
                            start=True, stop=True,
                        )
                    # constant shift (cancels in the softmax ratio) keeps the
                    # largest exp values small; softmax scale applied here in
                    # f32 rather than folded into quantized weights
                    e = epool.tile([128, 1024], bf16, tag="e")
                    nc.scalar.activation(
                        out=e, in_=ps,
                        func=mybir.ActivationFunctionType.Exp,
                        scale=SCALE, bias=eshift[:, 0:1],
                    )
                    e_tiles.append(e)
                return e_tiles

            def emit_attnv(s, b, ic, e_tiles, last=False):
                """attn@V + normalize + transpose into ot_sb.  For the
                final pre-collective block the normalize multiplies run on
                the (by then idle) activation engine, shortening the
                cross-engine latency chain ahead of the exposed AllToAll."""
                ps_o = ps_o_pool.tile([128, 4, 65], f32, tag="ps_o")
                for qt in range(4):
                    for jt in range(N_JT):
                        e = e_tiles[jt // 2]
                        qoff = (jt % 2) * 512 + qt * 128
                        nc.tensor.matmul(
                            ps_o[:, qt, :],
                            lhsT=e[:, qoff:qoff + 128],
                            rhs=vp_sb[:, b, jt, s, :],
                            start=(jt == 0), stop=(jt == N_JT - 1),
                        )
                r = rpool.tile([128, 4, 1], f32, tag="r")
                nc.vector.reciprocal(out=r, in_=ps_o[:, :, 64:65])
                for qt in range(4):
                    o_t = opool.tile([128, 64], bf16, tag="o")
                    if last:
                        nc.scalar.activation(
                            out=o_t, in_=ps_o[:, qt, 0:64],
                            func=mybir.ActivationFunctionType.Copy,
                            scale=r[:, qt, :],
                        )
                    else:
                        nc.vector.tensor_scalar_mul(
                            out=o_t, in0=ps_o[:, qt, 0:64], scalar1=r[:, qt, :],
                        )
                    tp = misc.tile([64, 128], bf16, tag="mpsum")
                    nc.tensor.transpose(tp, in_=o_t, identity=ident)
                    nc.vector.tensor_copy(
                        out=ot_sb[:, s, b,
                                  ic * 512 + qt * 128:ic * 512 + (qt + 1) * 128],
                        in_=tp,
                    )

            # partial projection accumulator (head-1 od chunks, + bias)
            y_acc = persist.tile([128, 4, C], f32)

            at_in = [dram.tile([512, 512], bf16, name=f"at_in{s}")
                     for s in range(HPC)]
            at_out = [dram.tile([512, 512], bf16, name=f"at_out{s}")
                      for s in range(HPC)]

            def emit_at_slice(s, b, ic):
                """ship one completed (head, batch, token-block) o^T slice
                into the AllToAll staging buffer as soon as it exists, so
                only a 64KB slice DMA precedes the final collective."""
                nc.sync.dma_start(
                    out=at_in[s].rearrange("(r p) t -> p r t", p=64)[
                        :, 4 * b + ic, :],
                    in_=ot_sb[:, s, b, ic * 512:(ic + 1) * 512],
                )

            def emit_at(s):
                """8-way AllToAll of head s's o^T + landing DMA into ogt.
                Head 0's landing is chunked per od so the final projection's
                first accumulation chunk starts as early as possible."""
                nc.gpsimd.collective_compute(
                    "AllToAll",
                    mybir.AluOpType.bypass,
                    ins=[at_in[s][:].opt()],
                    outs=[at_out[s][:].opt()],
                    replica_groups=GROUPS8,
                )
                view = at_out[s].rearrange("(o p) t -> p o t", p=128)
                if s == 0:
                    for o in range(4):
                        nc.sync.dma_start(out=ogt_sb[:, o, :],
                                          in_=view[:, o, :])
                else:
                    nc.sync.dma_start(
                        out=ogt_sb[:, s * 4:(s + 1) * 4, :], in_=view)

            def emit_partial_proj(tt):
                """proj over head-1 od chunks (4..7) into y_acc, + bias."""
                for nc2 in range(2):
                    ps = misc.tile([128, 512], f32, tag="mpsum",
                                   name=f"ps_pp{tt}_{nc2}")
                    for i, od in enumerate(range(4, 8)):
                        nc.tensor.matmul(
                            ps,
                            lhsT=ogt_sb[:, od, tt * 128:(tt + 1) * 128],
                            rhs=wp_sb[:, od, nc2 * 512:(nc2 + 1) * 512],
                            start=(i == 0), stop=(i == 3),
                        )
                    nc.vector.tensor_add(
                        out=y_acc[:, tt, nc2 * 512:(nc2 + 1) * 512],
                        in0=ps,
                        in1=bp_bc[:, nc2 * 512:(nc2 + 1) * 512],
                    )

            # ---------------- emission schedule ----------------
            # Unit order: (h0,b0), (h1,b0), (h1,b1), (h0,b1).
            # AT(h1) fires after unit 2 and hides under unit 3; partial
            # projection of the h1 od-chunks runs inside unit 3's blocks;
            # only AT(h0) + the h0 half of proj remain at the end.
            units = [(0, 0), (1, 0), (1, 1), (0, 1)]
            # per-(unit,ic) PE work interleaved into the blocks.  "pre"
            # slots produce data the NEXT block's scores need (q segments)
            # and run before the lookahead scores; "post" slots (v, weight
            # DMAs, ...) run after them, before attnV of the current block.
            # a pre-slot delays the next block's scores+exp, so only the
            # first q segment (needed by the immediately following lookahead)
            # lives there; everything else is emitted post-scores.
            slots_pre = {
                (0, 0): [lambda: emit_qk(0, "q", [1])],
            }
            slots_post = {
                # v(b0) must be fully emitted before unit 0's first attnV.
                # x(b1) load goes after the last emitted x(b0) reader
                # (q(b0) seg3, the (0,2) pre-slot).
                (0, 0): [lambda: emit_qk(0, "q", [2]),
                         lambda: emit_v(0, range(8))],
                (0, 1): [lambda: emit_qk(0, "q", [3])],
                (0, 2): [lambda: emit_xt_dma(1)],
                (0, 3): [lambda: emit_qk(1, "k", [0])],
                (1, 0): [lambda: emit_qk(1, "k", [1]),
                         lambda: nc.sync.dma_start(
                             out=wp_sb,
                             in_=wp_t.rearrange("(od p) c -> p od c", p=128))],
                (1, 1): [lambda: emit_qk(1, "k", [2]),
                         lambda: emit_v(1, [0]),
                         lambda: nc.sync.dma_start(
                             out=bp_bc,
                             in_=bass.AP(tensor=bp.tensor, offset=bp.offset,
                                         ap=[[0, 128]] + bp.ap))],
                (1, 2): [lambda: emit_qk(1, "k", [3]),
                         lambda: emit_qk(1, "q", [0]),
                         lambda: emit_v(1, [1])],
                (1, 3): [lambda: emit_qk(1, "q", [1]),
                         lambda: emit_v(1, [2, 3])],
                (2, 0): [lambda: emit_qk(1, "q", [2]),
                         lambda: emit_v(1, range(4, 8))],
                (2, 1): [lambda: emit_qk(1, "q", [3])],
            }

            # fill warmers: junk matmuls interleaved into the DMA-chased
            # k(b0) projection keep the PE busy-streak alive, so the whole
            # fill and the first attention block dispatch at full p-state
            # instead of the mid-clock ramp (the ps_s pool is idle here)
            wfill = ps_s_pool.tile([128, 1024], f32, tag="ps_s", name="wfill")

            def fill_warm(n):
                for _ in range(n):
                    nc.tensor.matmul(wfill[:, 0:128], lhsT=ident,
                                     rhs=bv_bc[:, 0, :, :],
                                     start=True, stop=True)

            fill_warm(12)
            for seg in range(4):
                ps = misc.tile([128, 512], f32, tag="mpsum")
                for ct in range(N_CT):
                    nc.tensor.matmul(
                        ps,
                        lhsT=wk_sb[:, ct, :],
                        rhs=xt_sb[:, ct, seg * 512:(seg + 1) * 512],
                        start=(ct == 0), stop=(ct == N_CT - 1),
                    )
                    if seg == 0:
                        fill_warm(10)
                nc.vector.tensor_scalar_add(
                    out=kt_sb[:, 0, seg * 512:(seg + 1) * 512],
                    in0=ps,
                    scalar1=bqk_sb[:, 1:2],
                )
            emit_qk(0, "q", [0])

            # flat block list, software-pipelined one block ahead: scores+exp
            # for block j+1 are emitted before attnV of block j, so the
            # activation engine never waits out attnV/transposes at block and
            # unit boundaries.
            blocks = [(ui, s, b, ic)
                      for ui, (s, b) in enumerate(units)
                      for ic in range(N_IC)]
            e_cur = emit_scores_exp(*blocks[0][1:])
            for j, (ui, s, b, ic) in enumerate(blocks):
                for work in slots_pre.get((ui, ic), []):
                    work()
                if j + 1 < len(blocks):
                    e_next = emit_scores_exp(*blocks[j + 1][1:])
                for work in slots_post.get((ui, ic), []):
                    work()
                emit_attnv(s, b, ic, e_cur, last=(j == len(blocks) - 1))
                emit_at_slice(s, b, ic)
                e_cur = e_next
                if ic == N_IC - 1:
                    if ui == 2:
                        emit_at(1)
                    elif ui == 3:
                        emit_at(0)

            # partial projection over the h1 od chunks (landed with AT(1))
            # overlaps AT(0)'s transfer
            for tt in range(TOKS // 128):
                emit_partial_proj(tt)

            # PE warmer: keep the tensor engine clocked up through the AT(0)
            # wait so the final projection issues at full p-state instead of
            # restarting from the low-clock ramp after ~25us of idle.
            warm = misc.tile([128, 512], f32, tag="mpsum", name="warm")
            for _ in range(76):
                nc.tensor.matmul(warm, lhsT=ogt_sb[:, 4, 0:128],
                                 rhs=wp_sb[:, 4, 0:512],
                                 start=True, stop=True)

            # ---------------- final projection (head-0 od chunks) ----------
            # runs after AT(0) lands; uses the attention-phase ps_s pool
            # (free by now) so each token tile needs one psum tile and one
            # wide DVE add
            for tt in range(TOKS // 128):
                ps_y = ps_s_pool.tile([128, 1024], f32, tag="ps_s",
                                      name=f"ps_y{tt}")
                for nc2 in range(2):
                    for od in range(4):
                        nc.tensor.matmul(
                            ps_y[:, nc2 * 512:(nc2 + 1) * 512],
                            lhsT=ogt_sb[:, od, tt * 128:(tt + 1) * 128],
                            rhs=wp_sb[:, od, nc2 * 512:(nc2 + 1) * 512],
                            start=(od == 0), stop=(od == 3),
                        )
                y_sb = ypool.tile([128, C], bf16, tag="y_sb")
                nc.vector.tensor_add(
                    out=y_sb, in0=ps_y, in1=y_acc[:, tt, :],
                )
                nc.sync.dma_start(out=y[tt * 128:(tt + 1) * 128, :], in_=y_sb)

    nc.compile()
    return nc


_CACHE = {}


def _get_nc():
    if "nc" not in _CACHE:
        _CACHE["nc"] = build_kernel()
    return _CACHE["nc"]


def make_in_maps(x, W_qkv, b_qkv, W_proj, b_proj):
    x = np.asarray(x, dtype=np.float32)
    W_qkv = np.asarray(W_qkv, dtype=np.float32)
    b_qkv = np.asarray(b_qkv, dtype=np.float32)
    W_proj = np.asarray(W_proj, dtype=np.float32)
    b_proj = np.asarray(b_proj, dtype=np.float32)

    Wq = W_qkv[0:C]
    Wk = W_qkv[C:2 * C]
    Wv = W_qkv[2 * C:3 * C]
    bq = b_qkv[0:C]
    bk = b_qkv[C:2 * C]
    bv_full = b_qkv[2 * C:3 * C]

    # x^T for both batches, shared by all cores
    xtb = np.ascontiguousarray(
        x.transpose(0, 2, 1)).astype(np_bf16)  # [B, C, N]

    # W_proj.T with rows permuted to the AllToAll arrival order:
    # od chunk (s, o) holds heads {4o+s, 4o+2+s}
    perm = []
    for s in range(HPC):
        for o in range(4):
            for h in (4 * o + s, 4 * o + 2 + s):
                perm.extend(range(64 * h, 64 * (h + 1)))
    wp_t_full = np.ascontiguousarray(W_proj.T[perm, :]).astype(np_bf16)

    in_maps = []
    for core in range(NCORES):
        rows = slice(128 * core, 128 * (core + 1))  # dims of heads 2c, 2c+1
        # fused [k|q|v] weight slab in [ct, 128, 384] layout
        slab = np.concatenate(
            [Wk[rows].T, Wq[rows].T, Wv[rows].T], axis=1)  # [C, 384]
        slab = np.ascontiguousarray(
            slab.reshape(N_CT, 128, 3 * 128)).astype(np_bf16)
        in_maps.append({
            "xt": xtb,
            "wkqv": slab,
            "bqk": np.ascontiguousarray(
                np.stack([bq[rows], bk[rows]], axis=1)),
            "bv": bv_full[rows].astype(np_bf16),
            "wp_t": wp_t_full,
            "bp": b_proj,
        })
    return in_maps


def kernel(x, W_qkv, b_qkv, W_proj, b_proj):
    nc = _get_nc()
    in_maps = make_in_maps(x, W_qkv, b_qkv, W_proj, b_proj)
    res = run_bass_kernel_spmd(nc, in_maps, list(range(NCORES)))

    out = np.empty((B, N, C), dtype=np.float32)
    for core in range(NCORES):
        b = core // 4
        g = core % 4
        out[b, g * TOKS:(g + 1) * TOKS, :] = res.results[core][
            "y"].astype(np.float32)
    return out
